# revision 9
# baseline (speedup 1.0000x reference)
"""Trainium2 Bass kernel for multi-head cross-attention.

Reference computation (fp32):
  q = x @ Wq; k = ctx @ Wk; v = ctx @ Wv              (per batch)
  sim = einsum('bihd,bjhd->bhij', q, k) * 1/sqrt(64)
  out = softmax(sim) @ v ; out = out @ Wo + bo

Shapes: x (4, 2048, 1024), context (4, 2048, 768), HEADS=8, DIM_HEAD=64.

Sharding: 8 cores = (batch b = core//2) x (query half = core%2). Each core
computes the full attention for its 1024 query rows across all 8 heads with
replicated weights; outputs concatenate - no cross-core reduction.

On-core dataflow (v2, pair-major, ACT-paced):
  - Heads are processed in PAIRS (2t, 2t+1) sharing feature tile t: the even
    head lives on SBUF partitions 0-63, the odd head on 64-127. The two QK^T
    matmuls of a pair have K=64 and are issued as row-tiled 64x128 PE tiles
    (tile_position (0,0) / (64,0)), so they run CONCURRENTLY - 2x throughput
    on the score phase.
  - Scores for (pair, i-half, jb, jb+1) land in one [128, 2048] PSUM tile
    (4 banks); ONE ACT exp (scale=1/8 folded) converts it to bf16 es - large
    ACT calls amortize the ~300-cycle ACTIVATE overhead (ACT is the pacing
    engine: 2048*1024*8 exps/core ~= 109us of ACT at 1 elem/lane/cycle).
  - PV accumulates per (pair, i-half): lhsT=[v_h|1] (65 cols: 64 dims +
    softmax-denominator ones column) into [65, 512] PSUM accs (1 bank each).
  - PSUM budget: S-quad 4 banks + 2 PV accs + 2 proj banks = 8 exactly. The
    spare 2 banks let projections for pair p+1 (Wq/Wk/Wv matmuls) interleave
    INTO pair p's attention loop, filling the PE while ACT paces the loop.
  - Normalize per pair off the PE: denominators staged via lane-shift DMAs
    into one [2, 1024] tile -> ONE batched DVE reciprocal (a [1,1024]
    single-lane reciprocal measures 6.5us on HW; batching pairs halves it),
    gpsimd partition_broadcast, DVE multiply. Odd head is lane-shifted into
    the stacked O^T layout via SBUF->SBUF DMA (DVE is lane-locked).
  - Final projection F = O^T.T @ Wo; bias is added on the DVE during PSUM
    evacuation from a gpsimd-broadcast [128, 1024] bias tile (no PE bias
    matmuls).
"""

import ml_dtypes
import numpy as np

import concourse.bass as bass
import concourse.tile as tile
from concourse import bacc, mybir
from concourse.bass_utils import run_bass_kernel_spmd

F32 = mybir.dt.float32
BF16 = mybir.dt.bfloat16

B = 4
NQ_FULL = 2048
NQ = 1024  # local query rows per core
NC = 2048
DQ = 1024
DC = 768
H = 8
DH = 64
INNER = H * DH  # 512
SCALE = DH ** -0.5

AT = DQ // 128   # 8  k-tiles of the q-projection contraction
BT = DC // 128   # 6  k-tiles of the k/v-projection contraction
CT = INNER // 128  # 4 feature tiles of q^T/k^T/o^T (= head pairs)
IB = NQ // 128   # 8  query-row blocks
JB = NC // 128   # 16 context-row blocks

_CACHE = {}


def _build_program():
    nc = bacc.Bacc(
        "TRN2",
        target_bir_lowering=False,
        debug=False,
        enable_asserts=False,
    )

    xT = nc.dram_tensor("xT", [DQ, NQ], BF16, kind="ExternalInput").ap()
    ctxT = nc.dram_tensor("ctxT", [DC, NC], BF16, kind="ExternalInput").ap()
    wq = nc.dram_tensor("Wq", [DQ, INNER], BF16, kind="ExternalInput").ap()
    wk = nc.dram_tensor("Wk", [DC, INNER], BF16, kind="ExternalInput").ap()
    wv = nc.dram_tensor("Wv", [DC, INNER], BF16, kind="ExternalInput").ap()
    wo = nc.dram_tensor("Wo", [INNER, DQ], BF16, kind="ExternalInput").ap()
    bo = nc.dram_tensor("bo", [DQ], BF16, kind="ExternalInput").ap()
    out = nc.dram_tensor("out", [NQ, DQ], F32, kind="ExternalOutput").ap()

    with tile.TileContext(nc) as tc:
        with nc.allow_low_precision(reason="bf16 matmul operands"):
            _emit(nc, tc, xT, ctxT, wq, wk, wv, wo, bo, out)

    nc.compile()
    return nc


def _emit(nc, tc, xT, ctxT, wq, wk, wv, wo, bo, out):
    from contextlib import ExitStack

    with ExitStack() as ctx:
        const = ctx.enter_context(tc.tile_pool(name="const", bufs=1))
        persist = ctx.enter_context(tc.tile_pool(name="persist", bufs=1))
        expp = ctx.enter_context(tc.tile_pool(name="expp", bufs=3))
        opool = ctx.enter_context(tc.tile_pool(name="opool", bufs=2))
        rpool = ctx.enter_context(tc.tile_pool(name="rpool", bufs=2))
        otmp = ctx.enter_context(tc.tile_pool(name="otmp", bufs=2))
        outp = ctx.enter_context(tc.tile_pool(name="outp", bufs=2))
        # PSUM: 8 banks total.  s-quad 4 + two PV accs 1+1 + proj 2 = 8.
        ps_s = ctx.enter_context(tc.tile_pool(name="ps_s", bufs=1, space="PSUM"))
        ps_acc = ctx.enter_context(tc.tile_pool(name="ps_acc", bufs=1, space="PSUM"))
        ps_pr = ctx.enter_context(tc.tile_pool(name="ps_pr", bufs=1, space="PSUM"))

        # --- constants ---
        bo_sb = const.tile([1, DQ], BF16)
        nc.sync.dma_start(out=bo_sb, in_=bo.unsqueeze(0))
        onesF = const.tile([128, 128], F32)
        nc.vector.memset(onesF, 1.0)
        bo_bc = const.tile([128, DQ], BF16)
        nc.gpsimd.partition_broadcast(bo_bc, bo_sb)

        # --- persistent SBUF tensors ---
        xT_sb = persist.tile([128, AT, NQ], BF16)      # 16 KB/part
        cx_sb = persist.tile([128, BT, NC], BF16)      # 24 KB/part
        wq_sb = persist.tile([128, AT, INNER], BF16)   # 8 KB
        wk_sb = persist.tile([128, BT, INNER], BF16)   # 6 KB
        wv_sb = persist.tile([128, BT, INNER], BF16)   # 6 KB
        wo_sb = persist.tile([128, CT, DQ], BF16)      # 8 KB
        qT_sb = persist.tile([128, CT, NQ], BF16)      # 8 KB
        kT_sb = persist.tile([128, CT, NC], BF16)      # 16 KB
        v_sb = persist.tile([128, JB, H * 65], BF16)   # 16.3 KB
        oT_sb = persist.tile([128, CT, NQ], BF16)      # 8 KB

        v4 = v_sb.rearrange("p j (h e) -> p j h e", e=65)

        # --- input DMAs, in consumption order ---
        wkr = wk.rearrange("(t p) c -> p t c", p=128)
        wvr = wv.rearrange("(t p) c -> p t c", p=128)
        for b in range(BT):
            nc.sync.dma_start(out=wk_sb[:, b, :], in_=wkr[:, b, :])
        ctxTr = ctxT.rearrange("(t p) j -> p t j", p=128)
        for jq in range(4):
            nc.sync.dma_start(
                out=cx_sb[:, :, jq * 512:(jq + 1) * 512],
                in_=ctxTr[:, :, jq * 512:(jq + 1) * 512],
            )
        for b in range(BT):
            nc.sync.dma_start(out=wv_sb[:, b, :], in_=wvr[:, b, :])
        wqr = wq.rearrange("(t p) c -> p t c", p=128)
        xTr = xT.rearrange("(t p) i -> p t i", p=128)
        for a in range(AT):
            nc.sync.dma_start(out=wq_sb[:, a, :], in_=wqr[:, a, :])
            nc.sync.dma_start(out=xT_sb[:, a, :], in_=xTr[:, a, :])
        wor = wo.rearrange("(t p) e -> p t e", p=128)
        for t in range(CT):
            nc.sync.dma_start(out=wo_sb[:, t, :], in_=wor[:, t, :])

        # ones columns of [v_h | 1]
        for jb in range(JB):
            nc.vector.tensor_copy(v4[:, jb, :, 64:65], onesF[:, 0:H].unsqueeze(-1))

        # ------------------------------------------------------------------
        # Projection work for head-pair p, as a list of (matmuls, evac)
        # closures so they can be interleaved into the previous pair's
        # attention loop.  Each group allocates one ps_pr tile.
        # ------------------------------------------------------------------
        def proj_groups(p):
            groups = []

            # k^T tile p:  k^T[c, j] via lhsT=Wk, rhs=ctx^T, per j-half
            def kproj(half):
                def run():
                    ps = ps_pr.tile([128, NQ], F32, tag="pr")
                    for b in range(BT):
                        for c2 in range(2):
                            sl = slice(half * 1024 + c2 * 512,
                                       half * 1024 + (c2 + 1) * 512)
                            nc.tensor.matmul(
                                ps[:, c2 * 512:(c2 + 1) * 512],
                                lhsT=wk_sb[:, b, p * 128:(p + 1) * 128],
                                rhs=cx_sb[:, b, sl],
                                start=(b == 0),
                                stop=(b == BT - 1),
                            )
                    nc.vector.tensor_copy(
                        kT_sb[:, p, half * 1024:(half + 1) * 1024], ps
                    )
                return run

            # q^T tile p
            def qproj():
                def run():
                    ps = ps_pr.tile([128, NQ], F32, tag="pr")
                    for a in range(AT):
                        for c2 in range(2):
                            nc.tensor.matmul(
                                ps[:, c2 * 512:(c2 + 1) * 512],
                                lhsT=wq_sb[:, a, p * 128:(p + 1) * 128],
                                rhs=xT_sb[:, a, c2 * 512:(c2 + 1) * 512],
                                start=(a == 0),
                                stop=(a == AT - 1),
                            )
                    nc.vector.tensor_copy(qT_sb[:, p, :], ps)
                return run

            # v columns for heads 2p, 2p+1: per jb, [128 j, 128 c]
            def vproj(jb):
                def run():
                    ps = ps_pr.tile([128, 128], F32, tag="pr")
                    for b in range(BT):
                        nc.tensor.matmul(
                            ps,
                            lhsT=cx_sb[:, b, jb * 128:(jb + 1) * 128],
                            rhs=wv_sb[:, b, p * 128:(p + 1) * 128],
                            start=(b == 0),
                            stop=(b == BT - 1),
                        )
                    nc.vector.tensor_copy(
                        v4[:, jb, 2 * p:2 * p + 2, 0:64],
                        ps.rearrange("p (h d) -> p h d", d=DH),
                    )
                return run

            groups.append(kproj(0))
            groups.append(kproj(1))
            groups.append(qproj())
            for jb in range(JB):
                groups.append(vproj(jb))
            return groups

        # ------------------------------------------------------------------
        # Attention for head-pair p (heads 2p / 2p+1), interleaving the
        # projection groups of pair p+1.
        # ------------------------------------------------------------------
        def attention(p, interleave):
            il = iter(interleave)
            n_emitted = 0

            def tick(budget):
                nonlocal n_emitted
                for _ in range(budget):
                    g = next(il, None)
                    if g is None:
                        return
                    g()
                    n_emitted += 1

            osb = {}
            for hh in range(2):
                osb[hh] = opool.tile(
                    [65, NQ], F32, tag=f"osb{hh}", name=f"osb{hh}"
                )
            dcol = rpool.tile([2, NQ], F32, tag="dcol")

            for ch in range(2):  # i-halves
                acc = {}
                for hh in range(2):
                    acc[hh] = ps_acc.tile(
                        [65, 512], F32, tag=f"acc{hh}", name=f"acc{hh}"
                    )
                for jj in range(JB // 2):  # jb pairs
                    jb0, jb1 = 2 * jj, 2 * jj + 1
                    sq = ps_s.tile([128, 2048], F32, tag="s")
                    for g, jb in ((0, jb0), (1, jb1)):
                        for hh in range(2):
                            po = 64 * hh
                            nc.tensor.matmul(
                                sq[:, (2 * g + hh) * 512:(2 * g + hh + 1) * 512],
                                lhsT=kT_sb[po:po + 64, p,
                                           jb * 128:(jb + 1) * 128],
                                rhs=qT_sb[po:po + 64, p,
                                          ch * 512:(ch + 1) * 512],
                                start=True,
                                stop=True,
                                tile_position=(po, 0),
                            )
                    es = expp.tile([128, 2048], BF16, tag="es")
                    nc.scalar.activation(
                        es, sq, mybir.ActivationFunctionType.Exp, scale=SCALE
                    )
                    for g, jb in ((0, jb0), (1, jb1)):
                        for hh in range(2):
                            nc.tensor.matmul(
                                acc[hh][0:65, :],
                                lhsT=v4[:, jb, 2 * p + hh, :],
                                rhs=es[:, (2 * g + hh) * 512:
                                       (2 * g + hh + 1) * 512],
                                start=(jb == 0),
                                stop=(jb == JB - 1),
                            )
                    tick(2)
                # evacuate accs -> osb halves (frees the acc banks)
                for hh in range(2):
                    nc.vector.tensor_copy(
                        osb[hh][:, ch * 512:(ch + 1) * 512], acc[hh]
                    )
            # drain any remaining interleave groups
            tick(1 << 30)

            # ---- normalize pair p (off the PE) ----
            # denominators (row 64 of osb) -> dcol rows 0/1 via lane-shift DMA
            for hh in range(2):
                nc.sync.dma_start(out=dcol[hh:hh + 1, :], in_=osb[hh][64:65, :])
            nc.vector.reciprocal_approx_fast(out=dcol, in_=dcol)
            r1 = rpool.tile([1, NQ], F32, tag="r1")
            nc.sync.dma_start(out=r1, in_=dcol[1:2, :])
            rb = {}
            rb[0] = rpool.tile([64, NQ], F32, tag="rb0", name="rb0")
            nc.gpsimd.partition_broadcast(rb[0], dcol[0:1, :])
            rb[1] = rpool.tile([64, NQ], F32, tag="rb1", name="rb1")
            nc.gpsimd.partition_broadcast(rb[1], r1)
            nc.vector.tensor_mul(oT_sb[0:64, p, :], osb[0][0:64, :], rb[0])
            ot = otmp.tile([64, NQ], BF16, tag="ot")
            nc.vector.tensor_mul(ot, osb[1][0:64, :], rb[1])
            nc.sync.dma_start(out=oT_sb[64:128, p, :], in_=ot)

        # ------------------------------------------------------------------
        # Emit: prelude projections for pair 0, then pair-major attention
        # with pair p+1's projections interleaved.
        # ------------------------------------------------------------------
        for g in proj_groups(0):
            g()
        for p in range(CT):
            interleave = proj_groups(p + 1) if p + 1 < CT else []
            attention(p, interleave)

        # --- output projection: F = O^T.T @ Wo;  bias added on DVE ---
        for pb in range(IB // 2):
            big = ps_s.tile([128, 2048], F32, tag="s")
            for half in range(2):
                ib = 2 * pb + half
                fp = big[:, half * 1024:(half + 1) * 1024]
                for c2 in range(2):
                    for t in range(CT):
                        nc.tensor.matmul(
                            fp[:, c2 * 512:(c2 + 1) * 512],
                            lhsT=oT_sb[:, t, ib * 128:(ib + 1) * 128],
                            rhs=wo_sb[:, t, c2 * 512:(c2 + 1) * 512],
                            start=(t == 0),
                            stop=(t == CT - 1),
                        )
                ost = outp.tile([128, DQ], F32)
                nc.vector.tensor_add(ost, fp, bo_bc)
                nc.sync.dma_start(out=out[ib * 128:(ib + 1) * 128, :], in_=ost)


def get_program():
    if "nc" not in _CACHE:
        _CACHE["nc"] = _build_program()
    return _CACHE["nc"]


def make_in_maps(x, context, Wq, Wk, Wv, Wo, bo):
    bf = ml_dtypes.bfloat16
    in_maps = []
    wq_b = np.asarray(Wq).astype(bf)
    wk_b = np.asarray(Wk).astype(bf)
    wv_b = np.asarray(Wv).astype(bf)
    wo_b = np.asarray(Wo).astype(bf)
    bo_b = np.asarray(bo).astype(bf)
    for c in range(8):
        b, half = c // 2, c % 2
        in_maps.append({
            "xT": np.ascontiguousarray(
                x[b, half * NQ:(half + 1) * NQ, :].T
            ).astype(bf),
            "ctxT": np.ascontiguousarray(context[b].T).astype(bf),
            "Wq": wq_b,
            "Wk": wk_b,
            "Wv": wv_b,
            "Wo": wo_b,
            "bo": bo_b,
        })
    return in_maps


def kernel(x, context, Wq, Wk, Wv, Wo, bo):
    nc = get_program()
    in_maps = make_in_maps(x, context, Wq, Wk, Wv, Wo, bo)
    res = run_bass_kernel_spmd(nc, in_maps, list(range(8)))
    out = np.empty((B, NQ_FULL, DQ), np.float32)
    for c in range(8):
        b, half = c // 2, c % 2
        out[b, half * NQ:(half + 1) * NQ, :] = res.results[c]["out"]
    return out


# revision 14
# speedup vs baseline: 1.4028x; 1.4028x over previous
"""Trainium2 Bass kernel for multi-head cross-attention.

Reference computation (fp32):
  q = x @ Wq; k = ctx @ Wk; v = ctx @ Wv              (per batch)
  sim = einsum('bihd,bjhd->bhij', q, k) * 1/sqrt(64)
  out = softmax(sim) @ v ; out = out @ Wo + bo

Shapes: x (4, 2048, 1024), context (4, 2048, 768), HEADS=8, DIM_HEAD=64.

Sharding: 8 cores = (batch b = core//2) x (query half = core%2). Each core
computes the full attention for its 1024 query rows across all 8 heads with
replicated weights; outputs concatenate - no cross-core reduction.

On-core dataflow (v2, pair-major, ACT-paced):
  - Heads are processed in PAIRS (2t, 2t+1) sharing feature tile t: the even
    head lives on SBUF partitions 0-63, the odd head on 64-127. The two QK^T
    matmuls of a pair have K=64 and are issued as row-tiled 64x128 PE tiles
    (tile_position (0,0) / (64,0)), so they run CONCURRENTLY - 2x throughput
    on the score phase.
  - Scores for (pair, i-half, jb, jb+1) land in one [128, 2048] PSUM tile
    (4 banks); ONE ACT exp (scale=1/8 folded) converts it to bf16 es - large
    ACT calls amortize the ~300-cycle ACTIVATE overhead (ACT is the pacing
    engine: 2048*1024*8 exps/core ~= 109us of ACT at 1 elem/lane/cycle).
  - PV accumulates per (pair, i-half): lhsT=[v_h|1] (65 cols: 64 dims +
    softmax-denominator ones column) into [65, 512] PSUM accs (1 bank each).
  - PSUM budget: S-quad 4 banks + 2 PV accs + 2 proj banks = 8 exactly. The
    spare 2 banks let projections for pair p+1 (Wq/Wk/Wv matmuls) interleave
    INTO pair p's attention loop, filling the PE while ACT paces the loop.
  - Normalize per pair off the PE: denominators staged via lane-shift DMAs
    into one [2, 1024] tile -> ONE batched DVE reciprocal (a [1,1024]
    single-lane reciprocal measures 6.5us on HW; batching pairs halves it),
    gpsimd partition_broadcast, DVE multiply. Odd head is lane-shifted into
    the stacked O^T layout via SBUF->SBUF DMA (DVE is lane-locked).
  - Final projection F = O^T.T @ Wo; bias is added on the DVE during PSUM
    evacuation from a gpsimd-broadcast [128, 1024] bias tile (no PE bias
    matmuls).
"""

import ml_dtypes
import numpy as np

import concourse.bass as bass
import concourse.tile as tile
from concourse import bacc, mybir
from concourse.bass_utils import run_bass_kernel_spmd

F32 = mybir.dt.float32
BF16 = mybir.dt.bfloat16

B = 4
NQ_FULL = 2048
NQ = 1024  # local query rows per core
NC = 2048
DQ = 1024
DC = 768
H = 8
DH = 64
INNER = H * DH  # 512
SCALE = DH ** -0.5

AT = DQ // 128   # 8  k-tiles of the q-projection contraction
BT = DC // 128   # 6  k-tiles of the k/v-projection contraction
CT = INNER // 128  # 4 feature tiles of q^T/k^T/o^T (= head pairs)
IB = NQ // 128   # 8  query-row blocks
JB = NC // 128   # 16 context-row blocks

_CACHE = {}


def _build_program():
    nc = bacc.Bacc(
        "TRN2",
        target_bir_lowering=False,
        debug=False,
        enable_asserts=False,
    )

    xT = nc.dram_tensor("xT", [DQ, NQ], BF16, kind="ExternalInput").ap()
    ctxT = nc.dram_tensor("ctxT", [DC, NC], BF16, kind="ExternalInput").ap()
    wq = nc.dram_tensor("Wq", [DQ, INNER], BF16, kind="ExternalInput").ap()
    wk = nc.dram_tensor("Wk", [DC, INNER], BF16, kind="ExternalInput").ap()
    wv = nc.dram_tensor("Wv", [DC, INNER], BF16, kind="ExternalInput").ap()
    wo = nc.dram_tensor("Wo", [INNER, DQ], BF16, kind="ExternalInput").ap()
    bo = nc.dram_tensor("bo", [DQ], BF16, kind="ExternalInput").ap()
    out = nc.dram_tensor("out", [NQ, DQ], F32, kind="ExternalOutput").ap()

    with tile.TileContext(nc) as tc:
        with nc.allow_low_precision(reason="bf16 matmul operands"):
            _emit(nc, tc, xT, ctxT, wq, wk, wv, wo, bo, out)

    nc.compile()
    return nc


def _emit(nc, tc, xT, ctxT, wq, wk, wv, wo, bo, out):
    from contextlib import ExitStack

    with ExitStack() as ctx:
        const = ctx.enter_context(tc.tile_pool(name="const", bufs=1))
        persist = ctx.enter_context(tc.tile_pool(name="persist", bufs=1))
        expp = ctx.enter_context(tc.tile_pool(name="expp", bufs=3))
        opool = ctx.enter_context(tc.tile_pool(name="opool", bufs=2))
        rpool = ctx.enter_context(tc.tile_pool(name="rpool", bufs=2))
        otmp = ctx.enter_context(tc.tile_pool(name="otmp", bufs=2))
        outp = ctx.enter_context(tc.tile_pool(name="outp", bufs=2))
        # PSUM: 8 banks total.  S tiles 2x2 + two PV accs 1+1 + proj 2 = 8.
        ps_s = ctx.enter_context(tc.tile_pool(name="ps_s", bufs=2, space="PSUM"))
        ps_acc = ctx.enter_context(tc.tile_pool(name="ps_acc", bufs=1, space="PSUM"))
        ps_pr = ctx.enter_context(tc.tile_pool(name="ps_pr", bufs=1, space="PSUM"))

        # --- constants ---
        bo_sb = const.tile([1, DQ], BF16)
        nc.sync.dma_start(out=bo_sb, in_=bo.unsqueeze(0))
        onesF = const.tile([128, 128], F32)
        nc.vector.memset(onesF, 1.0)
        bo_bc = const.tile([128, DQ], BF16)
        nc.gpsimd.partition_broadcast(bo_bc, bo_sb)

        # --- persistent SBUF tensors ---
        xT_sb = persist.tile([128, AT, NQ], BF16)      # 16 KB/part
        cx_sb = persist.tile([128, BT, NC], BF16)      # 24 KB/part
        wq_sb = persist.tile([128, AT, INNER], BF16)   # 8 KB
        wk_sb = persist.tile([128, BT, INNER], BF16)   # 6 KB
        wv_sb = persist.tile([128, BT, INNER], BF16)   # 6 KB
        wo_sb = persist.tile([128, CT, DQ], BF16)      # 8 KB
        qT_sb = persist.tile([128, CT, NQ], BF16)      # 8 KB
        kT_sb = persist.tile([128, CT, NC], BF16)      # 16 KB
        v_sb = persist.tile([128, JB, H * 65], BF16)   # 16.3 KB
        oT_sb = persist.tile([128, CT, NQ], BF16)      # 8 KB

        v4 = v_sb.rearrange("p j (h e) -> p j h e", e=65)

        # --- input DMAs, in consumption order ---
        wkr = wk.rearrange("(t p) c -> p t c", p=128)
        wvr = wv.rearrange("(t p) c -> p t c", p=128)
        for b in range(BT):
            nc.sync.dma_start(out=wk_sb[:, b, :], in_=wkr[:, b, :])
        ctxTr = ctxT.rearrange("(t p) j -> p t j", p=128)
        for jq in range(4):
            nc.sync.dma_start(
                out=cx_sb[:, :, jq * 512:(jq + 1) * 512],
                in_=ctxTr[:, :, jq * 512:(jq + 1) * 512],
            )
        for b in range(BT):
            nc.sync.dma_start(out=wv_sb[:, b, :], in_=wvr[:, b, :])
        wqr = wq.rearrange("(t p) c -> p t c", p=128)
        xTr = xT.rearrange("(t p) i -> p t i", p=128)
        for a in range(AT):
            nc.sync.dma_start(out=wq_sb[:, a, :], in_=wqr[:, a, :])
            nc.sync.dma_start(out=xT_sb[:, a, :], in_=xTr[:, a, :])
        wor = wo.rearrange("(t p) e -> p t e", p=128)
        for t in range(CT):
            nc.sync.dma_start(out=wo_sb[:, t, :], in_=wor[:, t, :])

        # ones columns of [v_h | 1]
        for jb in range(JB):
            nc.vector.tensor_copy(v4[:, jb, :, 64:65], onesF[:, 0:H].unsqueeze(-1))

        # ------------------------------------------------------------------
        # Projection work for head-pair p, as a list of (matmuls, evac)
        # closures so they can be interleaved into the previous pair's
        # attention loop.  Each group allocates one ps_pr tile.
        # ------------------------------------------------------------------
        def proj_groups(p):
            groups = []

            # k^T tile p:  k^T[c, j] via lhsT=Wk, rhs=ctx^T, per j-half
            def kproj(half):
                def run():
                    ps = ps_pr.tile([128, NQ], F32, tag="pr")
                    for b in range(BT):
                        for c2 in range(2):
                            sl = slice(half * 1024 + c2 * 512,
                                       half * 1024 + (c2 + 1) * 512)
                            nc.tensor.matmul(
                                ps[:, c2 * 512:(c2 + 1) * 512],
                                lhsT=wk_sb[:, b, p * 128:(p + 1) * 128],
                                rhs=cx_sb[:, b, sl],
                                start=(b == 0),
                                stop=(b == BT - 1),
                            )
                    nc.vector.tensor_copy(
                        kT_sb[:, p, half * 1024:(half + 1) * 1024], ps
                    )
                return run

            # q^T tile p
            def qproj():
                def run():
                    ps = ps_pr.tile([128, NQ], F32, tag="pr")
                    for a in range(AT):
                        for c2 in range(2):
                            nc.tensor.matmul(
                                ps[:, c2 * 512:(c2 + 1) * 512],
                                lhsT=wq_sb[:, a, p * 128:(p + 1) * 128],
                                rhs=xT_sb[:, a, c2 * 512:(c2 + 1) * 512],
                                start=(a == 0),
                                stop=(a == AT - 1),
                            )
                    nc.vector.tensor_copy(qT_sb[:, p, :], ps)
                return run

            groups.append(kproj(0))
            groups.append(kproj(1))
            groups.append(qproj())
            return groups

        # v columns for head-pairs [p0, p1): per jb, [128 j, (p1-p0)*128 c]
        def vproj_groups(p0, p1):
            w = (p1 - p0) * 128

            def vproj(jb):
                def run():
                    ps = ps_pr.tile([128, w], F32, tag="pr", name="vps")
                    for b in range(BT):
                        nc.tensor.matmul(
                            ps,
                            lhsT=cx_sb[:, b, jb * 128:(jb + 1) * 128],
                            rhs=wv_sb[:, b, p0 * 128:p1 * 128],
                            start=(b == 0),
                            stop=(b == BT - 1),
                        )
                    nc.vector.tensor_copy(
                        v4[:, jb, 2 * p0:2 * p1, 0:64],
                        ps.rearrange("p (h d) -> p h d", d=DH),
                    )
                return run

            return [vproj(jb) for jb in range(JB)]

        # ------------------------------------------------------------------
        # Attention for head-pair p (heads 2p / 2p+1), interleaving the
        # projection groups of pair p+1.
        # ------------------------------------------------------------------
        def attention(p, interleave):
            il = iter(interleave)
            n_emitted = 0

            def tick(budget):
                nonlocal n_emitted
                for _ in range(budget):
                    g = next(il, None)
                    if g is None:
                        return
                    g()
                    n_emitted += 1

            osb = {}
            for hh in range(2):
                osb[hh] = opool.tile(
                    [65, NQ], F32, tag=f"osb{hh}", name=f"osb{hh}"
                )
            dcol = rpool.tile([2, NQ], F32, tag="dcol")

            for ch in range(2):  # i-halves
                acc = {}
                for hh in range(2):
                    acc[hh] = ps_acc.tile(
                        [65, 512], F32, tag=f"acc{hh}", name=f"acc{hh}"
                    )
                for jb in range(JB):
                    sq = ps_s.tile([128, 1024], F32, tag="s")
                    for hh in range(2):
                        po = 64 * hh
                        nc.tensor.matmul(
                            sq[:, hh * 512:(hh + 1) * 512],
                            lhsT=kT_sb[po:po + 64, p,
                                       jb * 128:(jb + 1) * 128],
                            rhs=qT_sb[po:po + 64, p,
                                      ch * 512:(ch + 1) * 512],
                            start=True,
                            stop=True,
                            tile_position=(po, 0),
                        )
                    es = expp.tile([128, 1024], BF16, tag="es")
                    nc.scalar.activation(
                        es, sq, mybir.ActivationFunctionType.Exp, scale=SCALE
                    )
                    for hh in range(2):
                        nc.tensor.matmul(
                            acc[hh][0:65, :],
                            lhsT=v4[:, jb, 2 * p + hh, :],
                            rhs=es[:, hh * 512:(hh + 1) * 512],
                            start=(jb == 0),
                            stop=(jb == JB - 1),
                        )
                    tick(1)
                # evacuate accs -> osb halves (frees the acc banks)
                for hh in range(2):
                    nc.vector.tensor_copy(
                        osb[hh][:, ch * 512:(ch + 1) * 512], acc[hh]
                    )
            # drain any remaining interleave groups
            tick(1 << 30)

            # ---- normalize pair p (off the PE) ----
            # denominators (row 64 of osb) -> dcol rows 0/1 via lane-shift DMA
            for hh in range(2):
                nc.sync.dma_start(out=dcol[hh:hh + 1, :], in_=osb[hh][64:65, :])
            nc.vector.reciprocal_approx_fast(out=dcol, in_=dcol)
            r1 = rpool.tile([1, NQ], F32, tag="r1")
            nc.sync.dma_start(out=r1, in_=dcol[1:2, :])
            rb = {}
            rb[0] = rpool.tile([64, NQ], F32, tag="rb0", name="rb0")
            nc.gpsimd.partition_broadcast(rb[0], dcol[0:1, :])
            rb[1] = rpool.tile([64, NQ], F32, tag="rb1", name="rb1")
            nc.gpsimd.partition_broadcast(rb[1], r1)
            nc.vector.tensor_mul(oT_sb[0:64, p, :], osb[0][0:64, :], rb[0])
            ot = otmp.tile([64, NQ], BF16, tag="ot")
            nc.vector.tensor_mul(ot, osb[1][0:64, :], rb[1])
            nc.sync.dma_start(out=oT_sb[64:128, p, :], in_=ot)

        # ------------------------------------------------------------------
        # Emit: prelude projections for pair 0, then pair-major attention
        # with later pairs' projections interleaved (v for pair p+1 must
        # land during pair p; k^T/q^T for pair p+1 likewise).
        # ------------------------------------------------------------------
        for g in proj_groups(0):
            g()
        for g in vproj_groups(0, 1):
            g()
        inter = {
            0: proj_groups(1) + vproj_groups(1, 2),
            1: proj_groups(2) + vproj_groups(2, 4),
            2: proj_groups(3),
            3: [],
        }
        for p in range(CT):
            attention(p, inter[p])

        # --- output projection: F = O^T.T @ Wo;  bias added on DVE ---
        for ib in range(IB):
            fp = ps_s.tile([128, 1024], F32, tag="s", name="fp")
            for c2 in range(2):
                for t in range(CT):
                    nc.tensor.matmul(
                        fp[:, c2 * 512:(c2 + 1) * 512],
                        lhsT=oT_sb[:, t, ib * 128:(ib + 1) * 128],
                        rhs=wo_sb[:, t, c2 * 512:(c2 + 1) * 512],
                        start=(t == 0),
                        stop=(t == CT - 1),
                    )
            ost = outp.tile([128, DQ], F32)
            nc.vector.tensor_add(ost, fp, bo_bc)
            nc.sync.dma_start(out=out[ib * 128:(ib + 1) * 128, :], in_=ost)


def get_program():
    if "nc" not in _CACHE:
        _CACHE["nc"] = _build_program()
    return _CACHE["nc"]


def make_in_maps(x, context, Wq, Wk, Wv, Wo, bo):
    bf = ml_dtypes.bfloat16
    in_maps = []
    wq_b = np.asarray(Wq).astype(bf)
    wk_b = np.asarray(Wk).astype(bf)
    wv_b = np.asarray(Wv).astype(bf)
    wo_b = np.asarray(Wo).astype(bf)
    bo_b = np.asarray(bo).astype(bf)
    for c in range(8):
        b, half = c // 2, c % 2
        in_maps.append({
            "xT": np.ascontiguousarray(
                x[b, half * NQ:(half + 1) * NQ, :].T
            ).astype(bf),
            "ctxT": np.ascontiguousarray(context[b].T).astype(bf),
            "Wq": wq_b,
            "Wk": wk_b,
            "Wv": wv_b,
            "Wo": wo_b,
            "bo": bo_b,
        })
    return in_maps


def kernel(x, context, Wq, Wk, Wv, Wo, bo):
    nc = get_program()
    in_maps = make_in_maps(x, context, Wq, Wk, Wv, Wo, bo)
    res = run_bass_kernel_spmd(nc, in_maps, list(range(8)))
    out = np.empty((B, NQ_FULL, DQ), np.float32)
    for c in range(8):
        b, half = c // 2, c % 2
        out[b, half * NQ:(half + 1) * NQ, :] = res.results[c]["out"]
    return out


# revision 21
# speedup vs baseline: 1.4298x; 1.0192x over previous
"""Trainium2 Bass kernel for multi-head cross-attention.

Reference computation (fp32):
  q = x @ Wq; k = ctx @ Wk; v = ctx @ Wv              (per batch)
  sim = einsum('bihd,bjhd->bhij', q, k) * 1/sqrt(64)
  out = softmax(sim) @ v ; out = out @ Wo + bo

Shapes: x (4, 2048, 1024), context (4, 2048, 768), HEADS=8, DIM_HEAD=64.

Sharding: 8 cores = (batch b = core//2) x (query half = core%2). Each core
computes the full attention for its 1024 query rows across all 8 heads with
replicated weights; outputs concatenate - no cross-core reduction.

On-core dataflow (v2, pair-major, ACT-paced):
  - Heads are processed in PAIRS (2t, 2t+1) sharing feature tile t: the even
    head lives on SBUF partitions 0-63, the odd head on 64-127. The two QK^T
    matmuls of a pair have K=64 and are issued as row-tiled 64x128 PE tiles
    (tile_position (0,0) / (64,0)), so they run CONCURRENTLY - 2x throughput
    on the score phase.
  - Scores for (pair, i-half, jb, jb+1) land in one [128, 2048] PSUM tile
    (4 banks); ONE ACT exp (scale=1/8 folded) converts it to bf16 es - large
    ACT calls amortize the ~300-cycle ACTIVATE overhead (ACT is the pacing
    engine: 2048*1024*8 exps/core ~= 109us of ACT at 1 elem/lane/cycle).
  - PV accumulates per (pair, i-half): lhsT=[v_h|1] (65 cols: 64 dims +
    softmax-denominator ones column) into [65, 512] PSUM accs (1 bank each).
  - PSUM budget: S-quad 4 banks + 2 PV accs + 2 proj banks = 8 exactly. The
    spare 2 banks let projections for pair p+1 (Wq/Wk/Wv matmuls) interleave
    INTO pair p's attention loop, filling the PE while ACT paces the loop.
  - Normalize per pair off the PE: denominators staged via lane-shift DMAs
    into one [2, 1024] tile -> ONE batched DVE reciprocal (a [1,1024]
    single-lane reciprocal measures 6.5us on HW; batching pairs halves it),
    gpsimd partition_broadcast, DVE multiply. Odd head is lane-shifted into
    the stacked O^T layout via SBUF->SBUF DMA (DVE is lane-locked).
  - Final projection F = O^T.T @ Wo; bias is added on the DVE during PSUM
    evacuation from a gpsimd-broadcast [128, 1024] bias tile (no PE bias
    matmuls).
"""

import ml_dtypes
import numpy as np

import concourse.bass as bass
import concourse.tile as tile
from concourse import bacc, mybir
from concourse.bass_utils import run_bass_kernel_spmd

F32 = mybir.dt.float32
BF16 = mybir.dt.bfloat16

B = 4
NQ_FULL = 2048
NQ = 1024  # local query rows per core
NC = 2048
DQ = 1024
DC = 768
H = 8
DH = 64
INNER = H * DH  # 512
SCALE = DH ** -0.5

AT = DQ // 128   # 8  k-tiles of the q-projection contraction
BT = DC // 128   # 6  k-tiles of the k/v-projection contraction
CT = INNER // 128  # 4 feature tiles of q^T/k^T/o^T (= head pairs)
IB = NQ // 128   # 8  query-row blocks
JB = NC // 128   # 16 context-row blocks

_CACHE = {}


def _build_program():
    nc = bacc.Bacc(
        "TRN2",
        target_bir_lowering=False,
        debug=False,
        enable_asserts=False,
    )

    xT = nc.dram_tensor("xT", [DQ, NQ], BF16, kind="ExternalInput").ap()
    ctxT = nc.dram_tensor("ctxT", [DC, NC], BF16, kind="ExternalInput").ap()
    wq = nc.dram_tensor("Wq", [DQ, INNER], BF16, kind="ExternalInput").ap()
    wk = nc.dram_tensor("Wk", [DC, INNER], BF16, kind="ExternalInput").ap()
    wv = nc.dram_tensor("Wv", [DC, INNER], BF16, kind="ExternalInput").ap()
    wo = nc.dram_tensor("Wo", [INNER, DQ], BF16, kind="ExternalInput").ap()
    bo = nc.dram_tensor("bo", [DQ], BF16, kind="ExternalInput").ap()
    out = nc.dram_tensor("out", [NQ, DQ], F32, kind="ExternalOutput").ap()

    with tile.TileContext(nc) as tc:
        with nc.allow_low_precision(reason="bf16 matmul operands"):
            _emit(nc, tc, xT, ctxT, wq, wk, wv, wo, bo, out)

    nc.compile()
    return nc


def _emit(nc, tc, xT, ctxT, wq, wk, wv, wo, bo, out):
    from contextlib import ExitStack

    with ExitStack() as ctx:
        const = ctx.enter_context(tc.tile_pool(name="const", bufs=1))
        persist = ctx.enter_context(tc.tile_pool(name="persist", bufs=1))
        expp = ctx.enter_context(tc.tile_pool(name="expp", bufs=4))
        opool = ctx.enter_context(tc.tile_pool(name="opool", bufs=1))
        rpool = ctx.enter_context(tc.tile_pool(name="rpool", bufs=1))
        otmp = ctx.enter_context(tc.tile_pool(name="otmp", bufs=1))
        outp = ctx.enter_context(tc.tile_pool(name="outp", bufs=2))
        # PSUM: 8 banks total.  S tiles 2x2 + two PV accs 1+1 + proj 2 = 8.
        ps_s = ctx.enter_context(tc.tile_pool(name="ps_s", bufs=2, space="PSUM"))
        ps_acc = ctx.enter_context(tc.tile_pool(name="ps_acc", bufs=1, space="PSUM"))
        ps_pr = ctx.enter_context(tc.tile_pool(name="ps_pr", bufs=1, space="PSUM"))

        # --- constants ---
        bo_sb = const.tile([1, DQ], BF16)
        nc.sync.dma_start(out=bo_sb, in_=bo.unsqueeze(0))
        onesF = const.tile([128, 128], F32)
        nc.vector.memset(onesF, 1.0)
        bo_bc = const.tile([128, DQ], BF16)
        nc.gpsimd.partition_broadcast(bo_bc, bo_sb)

        # --- persistent SBUF tensors ---
        xT_sb = persist.tile([128, AT, NQ], BF16)      # 16 KB/part
        cx_sb = persist.tile([128, BT, NC], BF16)      # 24 KB/part
        wq_sb = persist.tile([128, AT, INNER], BF16)   # 8 KB
        wk_sb = persist.tile([128, BT, INNER], BF16)   # 6 KB
        wv_sb = persist.tile([128, BT, INNER], BF16)   # 6 KB
        wo_sb = persist.tile([128, CT, DQ], BF16)      # 8 KB
        # Per-head zero-padded q^T / k^T: head h occupies its native 64
        # partitions (even: 0-63, odd: 64-127); the other 64 rows are zeroed
        # once.  This lets the QK^T matmuls run as full K=128 128x128-mode
        # matmuls - no 64-row PE tiling modes, so the whole kernel stays in
        # one PE mode (no mode-switch drains) and LDWEIGHTS pipelines behind
        # the previous matmul via the background weight buffer.
        qT_sb = persist.tile([128, H, NQ], BF16)       # 16 KB
        kT_sb = persist.tile([128, H, NC], BF16)       # 32 KB
        v_sb = persist.tile([128, JB, H * 65], BF16)   # 16.3 KB
        oT_sb = persist.tile([128, CT, NQ], BF16)      # 8 KB

        v4 = v_sb.rearrange("p j (h e) -> p j h e", e=65)

        # --- input DMAs, in consumption order ---
        wkr = wk.rearrange("(t p) c -> p t c", p=128)
        wvr = wv.rearrange("(t p) c -> p t c", p=128)
        for b in range(BT):
            nc.sync.dma_start(out=wk_sb[:, b, :], in_=wkr[:, b, :])
        ctxTr = ctxT.rearrange("(t p) j -> p t j", p=128)
        for jq in range(4):
            nc.sync.dma_start(
                out=cx_sb[:, :, jq * 512:(jq + 1) * 512],
                in_=ctxTr[:, :, jq * 512:(jq + 1) * 512],
            )
        for b in range(BT):
            nc.sync.dma_start(out=wv_sb[:, b, :], in_=wvr[:, b, :])
        wqr = wq.rearrange("(t p) c -> p t c", p=128)
        xTr = xT.rearrange("(t p) i -> p t i", p=128)
        for a in range(AT):
            nc.sync.dma_start(out=wq_sb[:, a, :], in_=wqr[:, a, :])
            nc.sync.dma_start(out=xT_sb[:, a, :], in_=xTr[:, a, :])
        wor = wo.rearrange("(t p) e -> p t e", p=128)
        for t in range(CT):
            nc.sync.dma_start(out=wo_sb[:, t, :], in_=wor[:, t, :])

        # ones columns of [v_h | 1]
        for jb in range(JB):
            nc.vector.tensor_copy(v4[:, jb, :, 64:65], onesF[:, 0:H].unsqueeze(-1))
        # zero the dead half of each zero-padded head tile
        for t in range(CT):
            nc.vector.memset(qT_sb[64:128, 2 * t, :], 0.0)
            nc.vector.memset(qT_sb[0:64, 2 * t + 1, :], 0.0)
            nc.vector.memset(kT_sb[64:128, 2 * t, :], 0.0)
            nc.vector.memset(kT_sb[0:64, 2 * t + 1, :], 0.0)

        # ------------------------------------------------------------------
        # Projection work for head-pair p, as a list of (matmuls, evac)
        # closures so they can be interleaved into the previous pair's
        # attention loop.  Each group allocates one ps_pr tile.
        # ------------------------------------------------------------------
        def proj_groups(p):
            groups = []

            # k^T tile p:  k^T[c, j] via lhsT=Wk, rhs=ctx^T, per j-half.
            # Evacuate each head's 64 rows into its zero-padded tile.
            def kproj(half):
                def run():
                    ps = ps_pr.tile([128, NQ], F32, tag="pr")
                    for b in range(BT):
                        for c2 in range(2):
                            sl = slice(half * 1024 + c2 * 512,
                                       half * 1024 + (c2 + 1) * 512)
                            nc.tensor.matmul(
                                ps[:, c2 * 512:(c2 + 1) * 512],
                                lhsT=wk_sb[:, b, p * 128:(p + 1) * 128],
                                rhs=cx_sb[:, b, sl],
                                start=(b == 0),
                                stop=(b == BT - 1),
                            )
                    sl = slice(half * 1024, (half + 1) * 1024)
                    nc.vector.tensor_copy(
                        kT_sb[0:64, 2 * p, sl], ps[0:64, :]
                    )
                    nc.vector.tensor_copy(
                        kT_sb[64:128, 2 * p + 1, sl], ps[64:128, :]
                    )
                return run

            # q^T tile p
            def qproj():
                def run():
                    ps = ps_pr.tile([128, NQ], F32, tag="pr")
                    for a in range(AT):
                        for c2 in range(2):
                            nc.tensor.matmul(
                                ps[:, c2 * 512:(c2 + 1) * 512],
                                lhsT=wq_sb[:, a, p * 128:(p + 1) * 128],
                                rhs=xT_sb[:, a, c2 * 512:(c2 + 1) * 512],
                                start=(a == 0),
                                stop=(a == AT - 1),
                            )
                    nc.vector.tensor_copy(qT_sb[0:64, 2 * p, :], ps[0:64, :])
                    nc.vector.tensor_copy(
                        qT_sb[64:128, 2 * p + 1, :], ps[64:128, :]
                    )
                return run

            groups.append(kproj(0))
            groups.append(kproj(1))
            groups.append(qproj())
            return groups

        # v columns for head-pairs [p0, p1): per jb, [128 j, (p1-p0)*128 c]
        def vproj_groups(p0, p1):
            w = (p1 - p0) * 128

            def vproj(jb):
                def run():
                    ps = ps_pr.tile([128, w], F32, tag="pr", name="vps")
                    for b in range(BT):
                        nc.tensor.matmul(
                            ps,
                            lhsT=cx_sb[:, b, jb * 128:(jb + 1) * 128],
                            rhs=wv_sb[:, b, p0 * 128:p1 * 128],
                            start=(b == 0),
                            stop=(b == BT - 1),
                        )
                    nc.vector.tensor_copy(
                        v4[:, jb, 2 * p0:2 * p1, 0:64],
                        ps.rearrange("p (h d) -> p h d", d=DH),
                    )
                return run

            return [vproj(jb) for jb in range(JB)]

        # ------------------------------------------------------------------
        # Attention for head-pair p (heads 2p / 2p+1), interleaving the
        # projection groups of pair p+1.
        # ------------------------------------------------------------------
        def attention(p, interleave):
            il = iter(interleave)
            n_emitted = 0

            def tick(budget):
                nonlocal n_emitted
                for _ in range(budget):
                    g = next(il, None)
                    if g is None:
                        return
                    g()
                    n_emitted += 1

            osb = {}
            for hh in range(2):
                osb[hh] = opool.tile(
                    [65, NQ], F32, tag=f"osb{hh}", name=f"osb{hh}"
                )
            dcol = rpool.tile([2, NQ], F32, tag="dcol")

            LAG = 2  # PV trails exp by LAG iters so the PE never waits on ACT

            for ch in range(2):  # i-halves
                acc = {}
                for hh in range(2):
                    acc[hh] = ps_acc.tile(
                        [65, 512], F32, tag=f"acc{hh}", name=f"acc{hh}"
                    )
                es_q = []

                def pv(jb, es):
                    for hh in range(2):
                        nc.tensor.matmul(
                            acc[hh][0:65, :],
                            lhsT=v4[:, jb, 2 * p + hh, :],
                            rhs=es[:, hh * 512:(hh + 1) * 512],
                            start=(jb == 0),
                            stop=(jb == JB - 1),
                        )

                for jb in range(JB):
                    tick(1)
                    sq = ps_s.tile([128, 1024], F32, tag="s")
                    for hh in range(2):
                        nc.tensor.matmul(
                            sq[:, hh * 512:(hh + 1) * 512],
                            lhsT=kT_sb[:, 2 * p + hh,
                                       jb * 128:(jb + 1) * 128],
                            rhs=qT_sb[:, 2 * p + hh,
                                      ch * 512:(ch + 1) * 512],
                            start=True,
                            stop=True,
                        )
                    es = expp.tile([128, 1024], BF16, tag="es")
                    nc.scalar.activation(
                        es, sq, mybir.ActivationFunctionType.Exp, scale=SCALE
                    )
                    es_q.append((jb, es))
                    if len(es_q) > LAG:
                        pv(*es_q.pop(0))
                for it in es_q:
                    pv(*it)
                # evacuate accs -> osb halves (frees the acc banks)
                for hh in range(2):
                    nc.vector.tensor_copy(
                        osb[hh][:, ch * 512:(ch + 1) * 512], acc[hh]
                    )
            # drain any remaining interleave groups
            tick(1 << 30)

            # ---- normalize pair p (off the PE) ----
            # denominators (row 64 of osb) -> dcol rows 0/1 via lane-shift DMA
            for hh in range(2):
                nc.sync.dma_start(out=dcol[hh:hh + 1, :], in_=osb[hh][64:65, :])
            nc.vector.reciprocal_approx_fast(out=dcol, in_=dcol)
            r1 = rpool.tile([1, NQ], F32, tag="r1")
            nc.sync.dma_start(out=r1, in_=dcol[1:2, :])
            rb = {}
            rb[0] = rpool.tile([64, NQ], F32, tag="rb0", name="rb0")
            nc.gpsimd.partition_broadcast(rb[0], dcol[0:1, :])
            rb[1] = rpool.tile([64, NQ], F32, tag="rb1", name="rb1")
            nc.gpsimd.partition_broadcast(rb[1], r1)
            nc.vector.tensor_mul(oT_sb[0:64, p, :], osb[0][0:64, :], rb[0])
            ot = otmp.tile([64, NQ], BF16, tag="ot")
            nc.vector.tensor_mul(ot, osb[1][0:64, :], rb[1])
            nc.sync.dma_start(out=oT_sb[64:128, p, :], in_=ot)

        # ------------------------------------------------------------------
        # Emit: prelude projections for pair 0, then pair-major attention
        # with later pairs' projections interleaved (v for pair p+1 must
        # land during pair p; k^T/q^T for pair p+1 likewise).
        # ------------------------------------------------------------------
        vall = vproj_groups(0, CT)  # all heads at once, N=512 matmuls
        for g in proj_groups(0):
            g()
        for g in vall[0:3]:
            g()
        inter = {
            0: vall[3:] + proj_groups(1),
            1: proj_groups(2),
            2: proj_groups(3),
            3: [],
        }
        for p in range(CT):
            attention(p, inter[p])

        # --- output projection: F = O^T.T @ Wo;  bias added on DVE ---
        for ib in range(IB):
            fp = ps_s.tile([128, 1024], F32, tag="s", name="fp")
            for c2 in range(2):
                for t in range(CT):
                    nc.tensor.matmul(
                        fp[:, c2 * 512:(c2 + 1) * 512],
                        lhsT=oT_sb[:, t, ib * 128:(ib + 1) * 128],
                        rhs=wo_sb[:, t, c2 * 512:(c2 + 1) * 512],
                        start=(t == 0),
                        stop=(t == CT - 1),
                    )
            ost = outp.tile([128, DQ], F32)
            nc.vector.tensor_add(ost, fp, bo_bc)
            nc.sync.dma_start(out=out[ib * 128:(ib + 1) * 128, :], in_=ost)


def get_program():
    if "nc" not in _CACHE:
        _CACHE["nc"] = _build_program()
    return _CACHE["nc"]


def make_in_maps(x, context, Wq, Wk, Wv, Wo, bo):
    bf = ml_dtypes.bfloat16
    in_maps = []
    wq_b = np.asarray(Wq).astype(bf)
    wk_b = np.asarray(Wk).astype(bf)
    wv_b = np.asarray(Wv).astype(bf)
    wo_b = np.asarray(Wo).astype(bf)
    bo_b = np.asarray(bo).astype(bf)
    for c in range(8):
        b, half = c // 2, c % 2
        in_maps.append({
            "xT": np.ascontiguousarray(
                x[b, half * NQ:(half + 1) * NQ, :].T
            ).astype(bf),
            "ctxT": np.ascontiguousarray(context[b].T).astype(bf),
            "Wq": wq_b,
            "Wk": wk_b,
            "Wv": wv_b,
            "Wo": wo_b,
            "bo": bo_b,
        })
    return in_maps


def kernel(x, context, Wq, Wk, Wv, Wo, bo):
    nc = get_program()
    in_maps = make_in_maps(x, context, Wq, Wk, Wv, Wo, bo)
    res = run_bass_kernel_spmd(nc, in_maps, list(range(8)))
    out = np.empty((B, NQ_FULL, DQ), np.float32)
    for c in range(8):
        b, half = c // 2, c % 2
        out[b, half * NQ:(half + 1) * NQ, :] = res.results[c]["out"]
    return out


# revision 26
# speedup vs baseline: 1.4640x; 1.0240x over previous
"""Trainium2 Bass kernel for multi-head cross-attention.

Reference computation (fp32):
  q = x @ Wq; k = ctx @ Wk; v = ctx @ Wv              (per batch)
  sim = einsum('bihd,bjhd->bhij', q, k) * 1/sqrt(64)
  out = softmax(sim) @ v ; out = out @ Wo + bo

Shapes: x (4, 2048, 1024), context (4, 2048, 768), HEADS=8, DIM_HEAD=64.

Sharding: 8 cores = (batch b = core//2) x (query half = core%2). Each core
computes the full attention for its 1024 query rows across all 8 heads with
replicated weights; outputs concatenate - no cross-core reduction.

On-core dataflow (v2, pair-major, ACT-paced):
  - Heads are processed in PAIRS (2t, 2t+1) sharing feature tile t: the even
    head lives on SBUF partitions 0-63, the odd head on 64-127. The two QK^T
    matmuls of a pair have K=64 and are issued as row-tiled 64x128 PE tiles
    (tile_position (0,0) / (64,0)), so they run CONCURRENTLY - 2x throughput
    on the score phase.
  - Scores for (pair, i-half, jb, jb+1) land in one [128, 2048] PSUM tile
    (4 banks); ONE ACT exp (scale=1/8 folded) converts it to bf16 es - large
    ACT calls amortize the ~300-cycle ACTIVATE overhead (ACT is the pacing
    engine: 2048*1024*8 exps/core ~= 109us of ACT at 1 elem/lane/cycle).
  - PV accumulates per (pair, i-half): lhsT=[v_h|1] (65 cols: 64 dims +
    softmax-denominator ones column) into [65, 512] PSUM accs (1 bank each).
  - PSUM budget: S-quad 4 banks + 2 PV accs + 2 proj banks = 8 exactly. The
    spare 2 banks let projections for pair p+1 (Wq/Wk/Wv matmuls) interleave
    INTO pair p's attention loop, filling the PE while ACT paces the loop.
  - Normalize per pair off the PE: denominators staged via lane-shift DMAs
    into one [2, 1024] tile -> ONE batched DVE reciprocal (a [1,1024]
    single-lane reciprocal measures 6.5us on HW; batching pairs halves it),
    gpsimd partition_broadcast, DVE multiply. Odd head is lane-shifted into
    the stacked O^T layout via SBUF->SBUF DMA (DVE is lane-locked).
  - Final projection F = O^T.T @ Wo; bias is added on the DVE during PSUM
    evacuation from a gpsimd-broadcast [128, 1024] bias tile (no PE bias
    matmuls).
"""

import ml_dtypes
import numpy as np

import concourse.bass as bass
import concourse.tile as tile
from concourse import bacc, mybir
from concourse.bass_utils import run_bass_kernel_spmd

F32 = mybir.dt.float32
BF16 = mybir.dt.bfloat16

B = 4
NQ_FULL = 2048
NQ = 1024  # local query rows per core
NC = 2048
DQ = 1024
DC = 768
H = 8
DH = 64
INNER = H * DH  # 512
SCALE = DH ** -0.5

AT = DQ // 128   # 8  k-tiles of the q-projection contraction
BT = DC // 128   # 6  k-tiles of the k/v-projection contraction
CT = INNER // 128  # 4 feature tiles of q^T/k^T/o^T (= head pairs)
IB = NQ // 128   # 8  query-row blocks
JB = NC // 128   # 16 context-row blocks

_CACHE = {}


def _build_program():
    nc = bacc.Bacc(
        "TRN2",
        target_bir_lowering=False,
        debug=False,
        enable_asserts=False,
    )

    xT = nc.dram_tensor("xT", [DQ, NQ], BF16, kind="ExternalInput").ap()
    ctxT = nc.dram_tensor("ctxT", [DC, NC], BF16, kind="ExternalInput").ap()
    wq = nc.dram_tensor("Wq", [DQ, INNER], BF16, kind="ExternalInput").ap()
    wk = nc.dram_tensor("Wk", [DC, INNER], BF16, kind="ExternalInput").ap()
    wv = nc.dram_tensor("Wv", [DC, INNER], BF16, kind="ExternalInput").ap()
    wo = nc.dram_tensor("Wo", [INNER, DQ], BF16, kind="ExternalInput").ap()
    bo = nc.dram_tensor("bo", [DQ], BF16, kind="ExternalInput").ap()
    out = nc.dram_tensor("out", [NQ, DQ], F32, kind="ExternalOutput").ap()

    with tile.TileContext(nc) as tc:
        with nc.allow_low_precision(reason="bf16 matmul operands"):
            _emit(nc, tc, xT, ctxT, wq, wk, wv, wo, bo, out)

    nc.compile()
    return nc


def _emit(nc, tc, xT, ctxT, wq, wk, wv, wo, bo, out):
    from contextlib import ExitStack

    with ExitStack() as ctx:
        const = ctx.enter_context(tc.tile_pool(name="const", bufs=1))
        persist = ctx.enter_context(tc.tile_pool(name="persist", bufs=1))
        expp = ctx.enter_context(tc.tile_pool(name="expp", bufs=4))
        opool = ctx.enter_context(tc.tile_pool(name="opool", bufs=1))
        rpool = ctx.enter_context(tc.tile_pool(name="rpool", bufs=1))
        otmp = ctx.enter_context(tc.tile_pool(name="otmp", bufs=1))
        outp = ctx.enter_context(tc.tile_pool(name="outp", bufs=2))
        # PSUM: 8 banks total.  S tiles 2x2 + two PV accs 1+1 + proj 2 = 8.
        ps_s = ctx.enter_context(tc.tile_pool(name="ps_s", bufs=2, space="PSUM"))
        ps_acc = ctx.enter_context(tc.tile_pool(name="ps_acc", bufs=1, space="PSUM"))
        ps_pr = ctx.enter_context(tc.tile_pool(name="ps_pr", bufs=1, space="PSUM"))

        # --- constants ---
        bo_sb = const.tile([1, DQ], BF16)
        nc.sync.dma_start(out=bo_sb, in_=bo.unsqueeze(0))
        onesF = const.tile([128, 128], F32)
        nc.vector.memset(onesF, 1.0)
        bo_bc = const.tile([128, DQ], BF16)
        nc.gpsimd.partition_broadcast(bo_bc, bo_sb)

        # --- persistent SBUF tensors ---
        xT_sb = persist.tile([128, AT, NQ], BF16)      # 16 KB/part
        cx_sb = persist.tile([128, BT, NC], BF16)      # 24 KB/part
        wq_sb = persist.tile([128, AT, INNER], BF16)   # 8 KB
        wk_sb = persist.tile([128, BT, INNER], BF16)   # 6 KB
        wv_sb = persist.tile([128, BT, INNER], BF16)   # 6 KB
        wo_sb = persist.tile([128, CT, DQ], BF16)      # 8 KB
        # Per-head zero-padded q^T / k^T: head h occupies its native 64
        # partitions (even: 0-63, odd: 64-127); the other 64 rows are zeroed
        # once.  This lets the QK^T matmuls run as full K=128 128x128-mode
        # matmuls - no 64-row PE tiling modes, so the whole kernel stays in
        # one PE mode (no mode-switch drains) and LDWEIGHTS pipelines behind
        # the previous matmul via the background weight buffer.
        qT_sb = persist.tile([128, H, NQ], BF16)       # 16 KB
        kT_sb = persist.tile([128, H, NC], BF16)       # 32 KB
        v_sb = persist.tile([128, JB, H * 65], BF16)   # 16.3 KB
        oT_sb = persist.tile([128, CT, NQ], BF16)      # 8 KB

        v4 = v_sb.rearrange("p j (h e) -> p j h e", e=65)

        # --- input DMAs, in consumption order ---
        wkr = wk.rearrange("(t p) c -> p t c", p=128)
        wvr = wv.rearrange("(t p) c -> p t c", p=128)
        for b in range(BT):
            nc.sync.dma_start(out=wk_sb[:, b, :], in_=wkr[:, b, :])
        ctxTr = ctxT.rearrange("(t p) j -> p t j", p=128)
        for jq in range(4):
            nc.sync.dma_start(
                out=cx_sb[:, :, jq * 512:(jq + 1) * 512],
                in_=ctxTr[:, :, jq * 512:(jq + 1) * 512],
            )
        wqr = wq.rearrange("(t p) c -> p t c", p=128)
        xTr = xT.rearrange("(t p) i -> p t i", p=128)
        for a in range(AT):
            nc.sync.dma_start(out=wq_sb[:, a, :], in_=wqr[:, a, :])
            nc.sync.dma_start(out=xT_sb[:, a, :], in_=xTr[:, a, :])
        for b in range(BT):
            nc.sync.dma_start(out=wv_sb[:, b, :], in_=wvr[:, b, :])
        wor = wo.rearrange("(t p) e -> p t e", p=128)
        for t in range(CT):
            nc.sync.dma_start(out=wo_sb[:, t, :], in_=wor[:, t, :])

        # ones columns of [v_h | 1]
        for jb in range(JB):
            nc.vector.tensor_copy(v4[:, jb, :, 64:65], onesF[:, 0:H].unsqueeze(-1))
        # zero the dead half of each zero-padded head tile
        for t in range(CT):
            nc.vector.memset(qT_sb[64:128, 2 * t, :], 0.0)
            nc.vector.memset(qT_sb[0:64, 2 * t + 1, :], 0.0)
            nc.vector.memset(kT_sb[64:128, 2 * t, :], 0.0)
            nc.vector.memset(kT_sb[0:64, 2 * t + 1, :], 0.0)

        # ------------------------------------------------------------------
        # Projection work for head-pair p, as a list of (matmuls, evac)
        # closures so they can be interleaved into the previous pair's
        # attention loop.  Each group allocates one ps_pr tile.
        # ------------------------------------------------------------------
        def proj_groups(p):
            groups = []

            # k^T tile p:  k^T[c, j] via lhsT=Wk, rhs=ctx^T, per j-quarter
            # (quarter granularity so the first S matmuls start as soon as
            # ctx quarter 0 lands).  Evacuate each head's 64 rows into its
            # zero-padded tile.
            def kproj(jq):
                def run():
                    ps = ps_pr.tile([128, 512], F32, tag="pr", name="kps")
                    sl = slice(jq * 512, (jq + 1) * 512)
                    for b in range(BT):
                        nc.tensor.matmul(
                            ps,
                            lhsT=wk_sb[:, b, p * 128:(p + 1) * 128],
                            rhs=cx_sb[:, b, sl],
                            start=(b == 0),
                            stop=(b == BT - 1),
                        )
                    nc.vector.tensor_copy(
                        kT_sb[0:64, 2 * p, sl], ps[0:64, :]
                    )
                    nc.vector.tensor_copy(
                        kT_sb[64:128, 2 * p + 1, sl], ps[64:128, :]
                    )
                return run

            # q^T tile p
            def qproj():
                def run():
                    ps = ps_pr.tile([128, NQ], F32, tag="pr")
                    for a in range(AT):
                        for c2 in range(2):
                            nc.tensor.matmul(
                                ps[:, c2 * 512:(c2 + 1) * 512],
                                lhsT=wq_sb[:, a, p * 128:(p + 1) * 128],
                                rhs=xT_sb[:, a, c2 * 512:(c2 + 1) * 512],
                                start=(a == 0),
                                stop=(a == AT - 1),
                            )
                    nc.vector.tensor_copy(qT_sb[0:64, 2 * p, :], ps[0:64, :])
                    nc.vector.tensor_copy(
                        qT_sb[64:128, 2 * p + 1, :], ps[64:128, :]
                    )
                return run

            for jq in range(4):
                groups.append(kproj(jq))
            groups.append(qproj())
            return groups

        # v columns for head-pairs [p0, p1): per jb, [128 j, (p1-p0)*128 c]
        def vproj_groups(p0, p1):
            w = (p1 - p0) * 128

            def vproj(jb):
                def run():
                    ps = ps_pr.tile([128, w], F32, tag="pr", name="vps")
                    for b in range(BT):
                        nc.tensor.matmul(
                            ps,
                            lhsT=cx_sb[:, b, jb * 128:(jb + 1) * 128],
                            rhs=wv_sb[:, b, p0 * 128:p1 * 128],
                            start=(b == 0),
                            stop=(b == BT - 1),
                        )
                    nc.vector.tensor_copy(
                        v4[:, jb, 2 * p0:2 * p1, 0:64],
                        ps.rearrange("p (h d) -> p h d", d=DH),
                    )
                return run

            return [vproj(jb) for jb in range(JB)]

        # ------------------------------------------------------------------
        # Attention for head-pair p (heads 2p / 2p+1), interleaving the
        # projection groups of pair p+1.
        # ------------------------------------------------------------------
        def attention(p, interleave):
            il = iter(interleave)
            n_emitted = 0

            def tick(budget):
                nonlocal n_emitted
                for _ in range(budget):
                    g = next(il, None)
                    if g is None:
                        return
                    g()
                    n_emitted += 1

            osb = {}
            for hh in range(2):
                osb[hh] = opool.tile(
                    [65, NQ], F32, tag=f"osb{hh}", name=f"osb{hh}"
                )
            dcol = rpool.tile([2, NQ], F32, tag="dcol")
            r1 = rpool.tile([1, NQ], F32, tag="r1")
            rb = {}
            rb[0] = rpool.tile([64, NQ], F32, tag="rb0", name="rb0")
            rb[1] = rpool.tile([64, NQ], F32, tag="rb1", name="rb1")
            ot = otmp.tile([64, NQ], BF16, tag="ot")

            LAG = 2  # PV trails exp by LAG iters so the PE never waits on ACT

            for ch in range(2):  # i-halves
                acc = {}
                for hh in range(2):
                    acc[hh] = ps_acc.tile(
                        [65, 512], F32, tag=f"acc{hh}", name=f"acc{hh}"
                    )
                es_q = []

                def pv(jb, es):
                    for hh in range(2):
                        nc.tensor.matmul(
                            acc[hh][0:65, :],
                            lhsT=v4[:, jb, 2 * p + hh, :],
                            rhs=es[:, hh * 512:(hh + 1) * 512],
                            start=(jb == 0),
                            stop=(jb == JB - 1),
                        )

                for jb in range(JB):
                    tick(1)
                    sq = ps_s.tile([128, 1024], F32, tag="s")
                    for hh in range(2):
                        nc.tensor.matmul(
                            sq[:, hh * 512:(hh + 1) * 512],
                            lhsT=kT_sb[:, 2 * p + hh,
                                       jb * 128:(jb + 1) * 128],
                            rhs=qT_sb[:, 2 * p + hh,
                                      ch * 512:(ch + 1) * 512],
                            start=True,
                            stop=True,
                        )
                    es = expp.tile([128, 1024], BF16, tag="es")
                    nc.scalar.activation(
                        es, sq, mybir.ActivationFunctionType.Exp, scale=SCALE
                    )
                    es_q.append((jb, es))
                    if len(es_q) > LAG:
                        pv(*es_q.pop(0))
                for it in es_q:
                    pv(*it)
                # ---- evacuate + normalize this i-half (off the PE; the
                # ch=0 chain overlaps the ch=1 attention) ----
                sl = slice(ch * 512, (ch + 1) * 512)
                for hh in range(2):
                    nc.vector.tensor_copy(osb[hh][:, sl], acc[hh])
                for hh in range(2):
                    nc.sync.dma_start(
                        out=dcol[hh:hh + 1, sl], in_=osb[hh][64:65, sl]
                    )
                nc.vector.reciprocal_approx_fast(
                    out=dcol[:, sl], in_=dcol[:, sl]
                )
                nc.sync.dma_start(out=r1[0:1, sl], in_=dcol[1:2, sl])
                nc.gpsimd.partition_broadcast(rb[0][:, sl], dcol[0:1, sl])
                nc.gpsimd.partition_broadcast(rb[1][:, sl], r1[0:1, sl])
                nc.vector.tensor_mul(
                    oT_sb[0:64, p, sl], osb[0][0:64, sl], rb[0][:, sl]
                )
                nc.vector.tensor_mul(ot[:, sl], osb[1][0:64, sl], rb[1][:, sl])
                nc.sync.dma_start(out=oT_sb[64:128, p, sl], in_=ot[:, sl])
            # drain any remaining interleave groups
            tick(1 << 30)

        # ------------------------------------------------------------------
        # Emit: prelude projections for pair 0, then pair-major attention
        # with later pairs' projections interleaved (v for pair p+1 must
        # land during pair p; k^T/q^T for pair p+1 likewise).
        # ------------------------------------------------------------------
        vall = vproj_groups(0, CT)  # all heads at once, N=512 matmuls
        for g in proj_groups(0):
            g()
        for g in vall[0:3]:
            g()
        inter = {
            0: vall[3:] + proj_groups(1),
            1: proj_groups(2),
            2: proj_groups(3),
            3: [],
        }
        for p in range(CT):
            attention(p, inter[p])

        # --- output projection: F = O^T.T @ Wo;  bias added on DVE ---
        for ib in range(IB):
            fp = ps_s.tile([128, 1024], F32, tag="s", name="fp")
            for c2 in range(2):
                for t in range(CT):
                    nc.tensor.matmul(
                        fp[:, c2 * 512:(c2 + 1) * 512],
                        lhsT=oT_sb[:, t, ib * 128:(ib + 1) * 128],
                        rhs=wo_sb[:, t, c2 * 512:(c2 + 1) * 512],
                        start=(t == 0),
                        stop=(t == CT - 1),
                    )
            ost = outp.tile([128, DQ], F32)
            nc.vector.tensor_add(ost, fp, bo_bc)
            nc.sync.dma_start(out=out[ib * 128:(ib + 1) * 128, :], in_=ost)


def get_program():
    if "nc" not in _CACHE:
        _CACHE["nc"] = _build_program()
    return _CACHE["nc"]


def make_in_maps(x, context, Wq, Wk, Wv, Wo, bo):
    bf = ml_dtypes.bfloat16
    in_maps = []
    wq_b = np.asarray(Wq).astype(bf)
    wk_b = np.asarray(Wk).astype(bf)
    wv_b = np.asarray(Wv).astype(bf)
    wo_b = np.asarray(Wo).astype(bf)
    bo_b = np.asarray(bo).astype(bf)
    for c in range(8):
        b, half = c // 2, c % 2
        in_maps.append({
            "xT": np.ascontiguousarray(
                x[b, half * NQ:(half + 1) * NQ, :].T
            ).astype(bf),
            "ctxT": np.ascontiguousarray(context[b].T).astype(bf),
            "Wq": wq_b,
            "Wk": wk_b,
            "Wv": wv_b,
            "Wo": wo_b,
            "bo": bo_b,
        })
    return in_maps


def kernel(x, context, Wq, Wk, Wv, Wo, bo):
    nc = get_program()
    in_maps = make_in_maps(x, context, Wq, Wk, Wv, Wo, bo)
    res = run_bass_kernel_spmd(nc, in_maps, list(range(8)))
    out = np.empty((B, NQ_FULL, DQ), np.float32)
    for c in range(8):
        b, half = c // 2, c % 2
        out[b, half * NQ:(half + 1) * NQ, :] = res.results[c]["out"]
    return out


# revision 27
# speedup vs baseline: 1.4723x; 1.0056x over previous
"""Trainium2 Bass kernel for multi-head cross-attention.

Reference computation (fp32):
  q = x @ Wq; k = ctx @ Wk; v = ctx @ Wv              (per batch)
  sim = einsum('bihd,bjhd->bhij', q, k) * 1/sqrt(64)
  out = softmax(sim) @ v ; out = out @ Wo + bo

Shapes: x (4, 2048, 1024), context (4, 2048, 768), HEADS=8, DIM_HEAD=64.

Sharding: 8 cores = (batch b = core//2) x (query half = core%2). Each core
computes the full attention for its 1024 query rows across all 8 heads with
replicated weights; outputs concatenate - no cross-core reduction.

On-core dataflow (v2, pair-major, ACT-paced):
  - Heads are processed in PAIRS (2t, 2t+1) sharing feature tile t: the even
    head lives on SBUF partitions 0-63, the odd head on 64-127. The two QK^T
    matmuls of a pair have K=64 and are issued as row-tiled 64x128 PE tiles
    (tile_position (0,0) / (64,0)), so they run CONCURRENTLY - 2x throughput
    on the score phase.
  - Scores for (pair, i-half, jb, jb+1) land in one [128, 2048] PSUM tile
    (4 banks); ONE ACT exp (scale=1/8 folded) converts it to bf16 es - large
    ACT calls amortize the ~300-cycle ACTIVATE overhead (ACT is the pacing
    engine: 2048*1024*8 exps/core ~= 109us of ACT at 1 elem/lane/cycle).
  - PV accumulates per (pair, i-half): lhsT=[v_h|1] (65 cols: 64 dims +
    softmax-denominator ones column) into [65, 512] PSUM accs (1 bank each).
  - PSUM budget: S-quad 4 banks + 2 PV accs + 2 proj banks = 8 exactly. The
    spare 2 banks let projections for pair p+1 (Wq/Wk/Wv matmuls) interleave
    INTO pair p's attention loop, filling the PE while ACT paces the loop.
  - Normalize per pair off the PE: denominators staged via lane-shift DMAs
    into one [2, 1024] tile -> ONE batched DVE reciprocal (a [1,1024]
    single-lane reciprocal measures 6.5us on HW; batching pairs halves it),
    gpsimd partition_broadcast, DVE multiply. Odd head is lane-shifted into
    the stacked O^T layout via SBUF->SBUF DMA (DVE is lane-locked).
  - Final projection F = O^T.T @ Wo; bias is added on the DVE during PSUM
    evacuation from a gpsimd-broadcast [128, 1024] bias tile (no PE bias
    matmuls).
"""

import ml_dtypes
import numpy as np

import concourse.bass as bass
import concourse.tile as tile
from concourse import bacc, mybir
from concourse.bass_utils import run_bass_kernel_spmd

F32 = mybir.dt.float32
BF16 = mybir.dt.bfloat16

B = 4
NQ_FULL = 2048
NQ = 1024  # local query rows per core
NC = 2048
DQ = 1024
DC = 768
H = 8
DH = 64
INNER = H * DH  # 512
SCALE = DH ** -0.5

AT = DQ // 128   # 8  k-tiles of the q-projection contraction
BT = DC // 128   # 6  k-tiles of the k/v-projection contraction
CT = INNER // 128  # 4 feature tiles of q^T/k^T/o^T (= head pairs)
IB = NQ // 128   # 8  query-row blocks
JB = NC // 128   # 16 context-row blocks

_CACHE = {}


def _build_program():
    nc = bacc.Bacc(
        "TRN2",
        target_bir_lowering=False,
        debug=False,
        enable_asserts=False,
    )

    xT = nc.dram_tensor("xT", [DQ, NQ], BF16, kind="ExternalInput").ap()
    ctxT = nc.dram_tensor("ctxT", [DC, NC], BF16, kind="ExternalInput").ap()
    wq = nc.dram_tensor("Wq", [DQ, INNER], BF16, kind="ExternalInput").ap()
    wk = nc.dram_tensor("Wk", [DC, INNER], BF16, kind="ExternalInput").ap()
    wv = nc.dram_tensor("Wv", [DC, INNER], BF16, kind="ExternalInput").ap()
    wo = nc.dram_tensor("Wo", [INNER, DQ], BF16, kind="ExternalInput").ap()
    bo = nc.dram_tensor("bo", [DQ], BF16, kind="ExternalInput").ap()
    out = nc.dram_tensor("out", [NQ, DQ], F32, kind="ExternalOutput").ap()

    with tile.TileContext(nc) as tc:
        with nc.allow_low_precision(reason="bf16 matmul operands"):
            _emit(nc, tc, xT, ctxT, wq, wk, wv, wo, bo, out)

    nc.compile()
    return nc


def _emit(nc, tc, xT, ctxT, wq, wk, wv, wo, bo, out):
    from contextlib import ExitStack

    with ExitStack() as ctx:
        const = ctx.enter_context(tc.tile_pool(name="const", bufs=1))
        persist = ctx.enter_context(tc.tile_pool(name="persist", bufs=1))
        expp = ctx.enter_context(tc.tile_pool(name="expp", bufs=4))
        opool = ctx.enter_context(tc.tile_pool(name="opool", bufs=1))
        rpool = ctx.enter_context(tc.tile_pool(name="rpool", bufs=1))
        otmp = ctx.enter_context(tc.tile_pool(name="otmp", bufs=1))
        outp = ctx.enter_context(tc.tile_pool(name="outp", bufs=2))
        # PSUM: 8 banks total.  S tiles 2x2 + two PV accs 1+1 + proj 2 = 8.
        ps_s = ctx.enter_context(tc.tile_pool(name="ps_s", bufs=2, space="PSUM"))
        ps_acc = ctx.enter_context(tc.tile_pool(name="ps_acc", bufs=1, space="PSUM"))
        ps_pr = ctx.enter_context(tc.tile_pool(name="ps_pr", bufs=1, space="PSUM"))

        # --- constants ---
        bo_sb = const.tile([1, DQ], BF16)
        nc.sync.dma_start(out=bo_sb, in_=bo.unsqueeze(0))
        onesF = const.tile([128, 128], F32)
        nc.vector.memset(onesF, 1.0)
        bo_bc = const.tile([128, DQ], BF16)
        nc.gpsimd.partition_broadcast(bo_bc, bo_sb)

        # --- persistent SBUF tensors ---
        xT_sb = persist.tile([128, AT, NQ], BF16)      # 16 KB/part
        cx_sb = persist.tile([128, BT, NC], BF16)      # 24 KB/part
        wq_sb = persist.tile([128, AT, INNER], BF16)   # 8 KB
        wk_sb = persist.tile([128, BT, INNER], BF16)   # 6 KB
        wv_sb = persist.tile([128, BT, INNER], BF16)   # 6 KB
        wo_sb = persist.tile([128, CT, DQ], BF16)      # 8 KB
        # Per-head zero-padded q^T / k^T: head h occupies its native 64
        # partitions (even: 0-63, odd: 64-127); the other 64 rows are zeroed
        # once.  This lets the QK^T matmuls run as full K=128 128x128-mode
        # matmuls - no 64-row PE tiling modes, so the whole kernel stays in
        # one PE mode (no mode-switch drains) and LDWEIGHTS pipelines behind
        # the previous matmul via the background weight buffer.
        qT_sb = persist.tile([128, H, NQ], BF16)       # 16 KB
        kT_sb = persist.tile([128, H, NC], BF16)       # 32 KB
        v_sb = persist.tile([128, JB, H * 65], BF16)   # 16.3 KB
        oT_sb = persist.tile([128, CT, NQ], BF16)      # 8 KB

        v4 = v_sb.rearrange("p j (h e) -> p j h e", e=65)

        # --- input DMAs, in consumption order ---
        wkr = wk.rearrange("(t p) c -> p t c", p=128)
        wvr = wv.rearrange("(t p) c -> p t c", p=128)
        for b in range(BT):
            nc.sync.dma_start(out=wk_sb[:, b, :], in_=wkr[:, b, :])
        ctxTr = ctxT.rearrange("(t p) j -> p t j", p=128)
        for jq in range(4):
            nc.sync.dma_start(
                out=cx_sb[:, :, jq * 512:(jq + 1) * 512],
                in_=ctxTr[:, :, jq * 512:(jq + 1) * 512],
            )
        for b in range(BT):
            nc.sync.dma_start(out=wv_sb[:, b, :], in_=wvr[:, b, :])
        wqr = wq.rearrange("(t p) c -> p t c", p=128)
        xTr = xT.rearrange("(t p) i -> p t i", p=128)
        for a in range(AT):
            nc.sync.dma_start(out=wq_sb[:, a, :], in_=wqr[:, a, :])
            nc.sync.dma_start(out=xT_sb[:, a, :], in_=xTr[:, a, :])
        wor = wo.rearrange("(t p) e -> p t e", p=128)
        for t in range(CT):
            nc.sync.dma_start(out=wo_sb[:, t, :], in_=wor[:, t, :])

        # ones columns of [v_h | 1]
        for jb in range(JB):
            nc.vector.tensor_copy(v4[:, jb, :, 64:65], onesF[:, 0:H].unsqueeze(-1))
        # zero the dead half of each zero-padded head tile
        for t in range(CT):
            nc.vector.memset(qT_sb[64:128, 2 * t, :], 0.0)
            nc.vector.memset(qT_sb[0:64, 2 * t + 1, :], 0.0)
            nc.vector.memset(kT_sb[64:128, 2 * t, :], 0.0)
            nc.vector.memset(kT_sb[0:64, 2 * t + 1, :], 0.0)

        # ------------------------------------------------------------------
        # Projection work for head-pair p, as a list of (matmuls, evac)
        # closures so they can be interleaved into the previous pair's
        # attention loop.  Each group allocates one ps_pr tile.
        # ------------------------------------------------------------------
        def proj_groups(p):
            groups = []

            # k^T tile p:  k^T[c, j] via lhsT=Wk, rhs=ctx^T, per j-quarter
            # (quarter granularity so the first S matmuls start as soon as
            # ctx quarter 0 lands).  Evacuate each head's 64 rows into its
            # zero-padded tile.
            def kproj(jq):
                def run():
                    ps = ps_pr.tile([128, 512], F32, tag="pr", name="kps")
                    sl = slice(jq * 512, (jq + 1) * 512)
                    for b in range(BT):
                        nc.tensor.matmul(
                            ps,
                            lhsT=wk_sb[:, b, p * 128:(p + 1) * 128],
                            rhs=cx_sb[:, b, sl],
                            start=(b == 0),
                            stop=(b == BT - 1),
                        )
                    nc.vector.tensor_copy(
                        kT_sb[0:64, 2 * p, sl], ps[0:64, :]
                    )
                    nc.vector.tensor_copy(
                        kT_sb[64:128, 2 * p + 1, sl], ps[64:128, :]
                    )
                return run

            # q^T tile p
            def qproj():
                def run():
                    ps = ps_pr.tile([128, NQ], F32, tag="pr")
                    for a in range(AT):
                        for c2 in range(2):
                            nc.tensor.matmul(
                                ps[:, c2 * 512:(c2 + 1) * 512],
                                lhsT=wq_sb[:, a, p * 128:(p + 1) * 128],
                                rhs=xT_sb[:, a, c2 * 512:(c2 + 1) * 512],
                                start=(a == 0),
                                stop=(a == AT - 1),
                            )
                    nc.vector.tensor_copy(qT_sb[0:64, 2 * p, :], ps[0:64, :])
                    nc.vector.tensor_copy(
                        qT_sb[64:128, 2 * p + 1, :], ps[64:128, :]
                    )
                return run

            for jq in range(4):
                groups.append(kproj(jq))
            groups.append(qproj())
            return groups

        # v columns for head-pairs [p0, p1): per jb, [128 j, (p1-p0)*128 c]
        def vproj_groups(p0, p1):
            w = (p1 - p0) * 128

            def vproj(jb):
                def run():
                    ps = ps_pr.tile([128, w], F32, tag="pr", name="vps")
                    for b in range(BT):
                        nc.tensor.matmul(
                            ps,
                            lhsT=cx_sb[:, b, jb * 128:(jb + 1) * 128],
                            rhs=wv_sb[:, b, p0 * 128:p1 * 128],
                            start=(b == 0),
                            stop=(b == BT - 1),
                        )
                    nc.vector.tensor_copy(
                        v4[:, jb, 2 * p0:2 * p1, 0:64],
                        ps.rearrange("p (h d) -> p h d", d=DH),
                    )
                return run

            return [vproj(jb) for jb in range(JB)]

        # ------------------------------------------------------------------
        # Attention for head-pair p (heads 2p / 2p+1), interleaving the
        # projection groups of pair p+1.
        # ------------------------------------------------------------------
        def attention(p, interleave):
            il = iter(interleave)
            n_emitted = 0

            def tick(budget):
                nonlocal n_emitted
                for _ in range(budget):
                    g = next(il, None)
                    if g is None:
                        return
                    g()
                    n_emitted += 1

            osb = {}
            for hh in range(2):
                osb[hh] = opool.tile(
                    [65, NQ], F32, tag=f"osb{hh}", name=f"osb{hh}"
                )
            dcol = rpool.tile([2, NQ], F32, tag="dcol")
            r1 = rpool.tile([1, NQ], F32, tag="r1")
            rb = {}
            rb[0] = rpool.tile([64, NQ], F32, tag="rb0", name="rb0")
            rb[1] = rpool.tile([64, NQ], F32, tag="rb1", name="rb1")
            ot = otmp.tile([64, NQ], BF16, tag="ot")

            LAG = 2  # PV trails exp by LAG iters so the PE never waits on ACT

            for ch in range(2):  # i-halves
                acc = {}
                for hh in range(2):
                    acc[hh] = ps_acc.tile(
                        [65, 512], F32, tag=f"acc{hh}", name=f"acc{hh}"
                    )
                es_q = []

                def pv(jb, es):
                    for hh in range(2):
                        nc.tensor.matmul(
                            acc[hh][0:65, :],
                            lhsT=v4[:, jb, 2 * p + hh, :],
                            rhs=es[:, hh * 512:(hh + 1) * 512],
                            start=(jb == 0),
                            stop=(jb == JB - 1),
                        )

                for jb in range(JB):
                    tick(1)
                    sq = ps_s.tile([128, 1024], F32, tag="s")
                    for hh in range(2):
                        nc.tensor.matmul(
                            sq[:, hh * 512:(hh + 1) * 512],
                            lhsT=kT_sb[:, 2 * p + hh,
                                       jb * 128:(jb + 1) * 128],
                            rhs=qT_sb[:, 2 * p + hh,
                                      ch * 512:(ch + 1) * 512],
                            start=True,
                            stop=True,
                        )
                    es = expp.tile([128, 1024], BF16, tag="es")
                    nc.scalar.activation(
                        es, sq, mybir.ActivationFunctionType.Exp, scale=SCALE
                    )
                    es_q.append((jb, es))
                    if len(es_q) > LAG:
                        pv(*es_q.pop(0))
                for it in es_q:
                    pv(*it)
                # ---- evacuate + normalize this i-half (off the PE; the
                # ch=0 chain overlaps the ch=1 attention) ----
                sl = slice(ch * 512, (ch + 1) * 512)
                for hh in range(2):
                    nc.vector.tensor_copy(osb[hh][:, sl], acc[hh])
                for hh in range(2):
                    nc.sync.dma_start(
                        out=dcol[hh:hh + 1, sl], in_=osb[hh][64:65, sl]
                    )
                nc.vector.reciprocal_approx_fast(
                    out=dcol[:, sl], in_=dcol[:, sl]
                )
                nc.sync.dma_start(out=r1[0:1, sl], in_=dcol[1:2, sl])
                nc.gpsimd.partition_broadcast(rb[0][:, sl], dcol[0:1, sl])
                nc.gpsimd.partition_broadcast(rb[1][:, sl], r1[0:1, sl])
                nc.vector.tensor_mul(
                    oT_sb[0:64, p, sl], osb[0][0:64, sl], rb[0][:, sl]
                )
                nc.vector.tensor_mul(ot[:, sl], osb[1][0:64, sl], rb[1][:, sl])
                nc.sync.dma_start(out=oT_sb[64:128, p, sl], in_=ot[:, sl])
            # drain any remaining interleave groups
            tick(1 << 30)

        # ------------------------------------------------------------------
        # Emit: prelude projections for pair 0, then pair-major attention
        # with later pairs' projections interleaved (v for pair p+1 must
        # land during pair p; k^T/q^T for pair p+1 likewise).
        # ------------------------------------------------------------------
        vall = vproj_groups(0, CT)  # all heads at once, N=512 matmuls
        for g in proj_groups(0):
            g()
        for g in vall[0:3]:
            g()
        inter = {
            0: vall[3:] + proj_groups(1),
            1: proj_groups(2),
            2: proj_groups(3),
            3: [],
        }
        for p in range(CT):
            attention(p, inter[p])

        # --- output projection: F = O^T.T @ Wo;  bias added on DVE ---
        for ib in range(IB):
            fp = ps_s.tile([128, 1024], F32, tag="s", name="fp")
            for c2 in range(2):
                for t in range(CT):
                    nc.tensor.matmul(
                        fp[:, c2 * 512:(c2 + 1) * 512],
                        lhsT=oT_sb[:, t, ib * 128:(ib + 1) * 128],
                        rhs=wo_sb[:, t, c2 * 512:(c2 + 1) * 512],
                        start=(t == 0),
                        stop=(t == CT - 1),
                    )
            ost = outp.tile([128, DQ], F32)
            nc.vector.tensor_add(ost, fp, bo_bc)
            nc.sync.dma_start(out=out[ib * 128:(ib + 1) * 128, :], in_=ost)


def get_program():
    if "nc" not in _CACHE:
        _CACHE["nc"] = _build_program()
    return _CACHE["nc"]


def make_in_maps(x, context, Wq, Wk, Wv, Wo, bo):
    bf = ml_dtypes.bfloat16
    in_maps = []
    wq_b = np.asarray(Wq).astype(bf)
    wk_b = np.asarray(Wk).astype(bf)
    wv_b = np.asarray(Wv).astype(bf)
    wo_b = np.asarray(Wo).astype(bf)
    bo_b = np.asarray(bo).astype(bf)
    for c in range(8):
        b, half = c // 2, c % 2
        in_maps.append({
            "xT": np.ascontiguousarray(
                x[b, half * NQ:(half + 1) * NQ, :].T
            ).astype(bf),
            "ctxT": np.ascontiguousarray(context[b].T).astype(bf),
            "Wq": wq_b,
            "Wk": wk_b,
            "Wv": wv_b,
            "Wo": wo_b,
            "bo": bo_b,
        })
    return in_maps


def kernel(x, context, Wq, Wk, Wv, Wo, bo):
    nc = get_program()
    in_maps = make_in_maps(x, context, Wq, Wk, Wv, Wo, bo)
    res = run_bass_kernel_spmd(nc, in_maps, list(range(8)))
    out = np.empty((B, NQ_FULL, DQ), np.float32)
    for c in range(8):
        b, half = c // 2, c % 2
        out[b, half * NQ:(half + 1) * NQ, :] = res.results[c]["out"]
    return out


# revision 34
# speedup vs baseline: 1.4991x; 1.0182x over previous
"""Trainium2 Bass kernel for multi-head cross-attention.

Reference computation (fp32):
  q = x @ Wq; k = ctx @ Wk; v = ctx @ Wv              (per batch)
  sim = einsum('bihd,bjhd->bhij', q, k) * 1/sqrt(64)
  out = softmax(sim) @ v ; out = out @ Wo + bo

Shapes: x (4, 2048, 1024), context (4, 2048, 768), HEADS=8, DIM_HEAD=64.

Sharding: 8 cores = (batch b = core//2) x (query half = core%2). Each core
computes the full attention for its 1024 query rows across all 8 heads with
replicated weights; outputs concatenate - no cross-core reduction.

On-core dataflow (v2, pair-major, ACT-paced):
  - Heads are processed in PAIRS (2t, 2t+1) sharing feature tile t: the even
    head lives on SBUF partitions 0-63, the odd head on 64-127. The two QK^T
    matmuls of a pair have K=64 and are issued as row-tiled 64x128 PE tiles
    (tile_position (0,0) / (64,0)), so they run CONCURRENTLY - 2x throughput
    on the score phase.
  - Scores for (pair, i-half, jb, jb+1) land in one [128, 2048] PSUM tile
    (4 banks); ONE ACT exp (scale=1/8 folded) converts it to bf16 es - large
    ACT calls amortize the ~300-cycle ACTIVATE overhead (ACT is the pacing
    engine: 2048*1024*8 exps/core ~= 109us of ACT at 1 elem/lane/cycle).
  - PV accumulates per (pair, i-half): lhsT=[v_h|1] (65 cols: 64 dims +
    softmax-denominator ones column) into [65, 512] PSUM accs (1 bank each).
  - PSUM budget: S-quad 4 banks + 2 PV accs + 2 proj banks = 8 exactly. The
    spare 2 banks let projections for pair p+1 (Wq/Wk/Wv matmuls) interleave
    INTO pair p's attention loop, filling the PE while ACT paces the loop.
  - Normalize per pair off the PE: denominators staged via lane-shift DMAs
    into one [2, 1024] tile -> ONE batched DVE reciprocal (a [1,1024]
    single-lane reciprocal measures 6.5us on HW; batching pairs halves it),
    gpsimd partition_broadcast, DVE multiply. Odd head is lane-shifted into
    the stacked O^T layout via SBUF->SBUF DMA (DVE is lane-locked).
  - Final projection F = O^T.T @ Wo; bias is added on the DVE during PSUM
    evacuation from a gpsimd-broadcast [128, 1024] bias tile (no PE bias
    matmuls).
"""

import ml_dtypes
import numpy as np

import concourse.bass as bass
import concourse.tile as tile
from concourse import bacc, mybir
from concourse.bass_utils import run_bass_kernel_spmd

F32 = mybir.dt.float32
BF16 = mybir.dt.bfloat16

B = 4
NQ_FULL = 2048
NQ = 1024  # local query rows per core
NC = 2048
DQ = 1024
DC = 768
H = 8
DH = 64
INNER = H * DH  # 512
SCALE = DH ** -0.5

AT = DQ // 128   # 8  k-tiles of the q-projection contraction
BT = DC // 128   # 6  k-tiles of the k/v-projection contraction
CT = INNER // 128  # 4 feature tiles of q^T/k^T/o^T (= head pairs)
IB = NQ // 128   # 8  query-row blocks
JB = NC // 128   # 16 context-row blocks

_CACHE = {}


def _build_program():
    nc = bacc.Bacc(
        "TRN2",
        target_bir_lowering=False,
        debug=False,
        enable_asserts=False,
    )

    xT = nc.dram_tensor("xT", [DQ, NQ], BF16, kind="ExternalInput").ap()
    ctxT = nc.dram_tensor("ctxT", [DC, NC], BF16, kind="ExternalInput").ap()
    wq = nc.dram_tensor("Wq", [DQ, INNER], BF16, kind="ExternalInput").ap()
    wk = nc.dram_tensor("Wk", [DC, INNER], BF16, kind="ExternalInput").ap()
    wv = nc.dram_tensor("Wv", [DC, INNER], BF16, kind="ExternalInput").ap()
    wo = nc.dram_tensor("Wo", [INNER, DQ], BF16, kind="ExternalInput").ap()
    bo = nc.dram_tensor("bo", [DQ], BF16, kind="ExternalInput").ap()
    out = nc.dram_tensor("out", [NQ, DQ], F32, kind="ExternalOutput").ap()

    with tile.TileContext(nc) as tc:
        with nc.allow_low_precision(reason="bf16 matmul operands"):
            _emit(nc, tc, xT, ctxT, wq, wk, wv, wo, bo, out)

    nc.compile()
    return nc


def _emit(nc, tc, xT, ctxT, wq, wk, wv, wo, bo, out):
    from contextlib import ExitStack

    with ExitStack() as ctx:
        const = ctx.enter_context(tc.tile_pool(name="const", bufs=1))
        persist = ctx.enter_context(tc.tile_pool(name="persist", bufs=1))
        expp = ctx.enter_context(tc.tile_pool(name="expp", bufs=4))
        opool = ctx.enter_context(tc.tile_pool(name="opool", bufs=1))
        rpool = ctx.enter_context(tc.tile_pool(name="rpool", bufs=1))
        otmp = ctx.enter_context(tc.tile_pool(name="otmp", bufs=1))
        outp = ctx.enter_context(tc.tile_pool(name="outp", bufs=2))
        # PSUM: 8 banks total.  S tiles 2x2 + two PV accs 1+1 + proj 2 = 8.
        ps_s = ctx.enter_context(tc.tile_pool(name="ps_s", bufs=2, space="PSUM"))
        ps_acc = ctx.enter_context(tc.tile_pool(name="ps_acc", bufs=1, space="PSUM"))
        ps_pr = ctx.enter_context(tc.tile_pool(name="ps_pr", bufs=1, space="PSUM"))

        # --- constants ---
        bo_sb = const.tile([1, DQ], BF16)
        nc.sync.dma_start(out=bo_sb, in_=bo.unsqueeze(0))
        onesF = const.tile([128, 128], F32)
        nc.vector.memset(onesF, 1.0)
        bo_bc = const.tile([128, DQ], BF16)
        nc.gpsimd.partition_broadcast(bo_bc, bo_sb)

        # --- persistent SBUF tensors ---
        xT_sb = persist.tile([128, AT, NQ], BF16)      # 16 KB/part
        cx_sb = persist.tile([128, BT, NC], BF16)      # 24 KB/part
        wq_sb = persist.tile([128, AT, INNER], BF16)   # 8 KB
        wk_sb = persist.tile([128, BT, INNER], BF16)   # 6 KB
        wv_sb = persist.tile([128, BT, INNER], BF16)   # 6 KB
        wo_sb = persist.tile([128, CT, DQ], BF16)      # 8 KB
        # Per-head zero-padded q^T / k^T: head h occupies its native 64
        # partitions (even: 0-63, odd: 64-127); the other 64 rows are zeroed
        # once.  This lets the QK^T matmuls run as full K=128 128x128-mode
        # matmuls - no 64-row PE tiling modes, so the whole kernel stays in
        # one PE mode (no mode-switch drains) and LDWEIGHTS pipelines behind
        # the previous matmul via the background weight buffer.
        qT_sb = persist.tile([128, H, NQ], BF16)       # 16 KB
        kT_sb = persist.tile([128, H, NC], BF16)       # 32 KB
        v_sb = persist.tile([128, JB, H * 65], BF16)   # 16.3 KB
        oT_sb = persist.tile([128, CT, NQ], BF16)      # 8 KB

        v4 = v_sb.rearrange("p j (h e) -> p j h e", e=65)

        # --- input DMAs, in consumption order ---
        wkr = wk.rearrange("(t p) c -> p t c", p=128)
        wvr = wv.rearrange("(t p) c -> p t c", p=128)
        for b in range(BT):
            nc.sync.dma_start(out=wk_sb[:, b, :], in_=wkr[:, b, :])
        ctxTr = ctxT.rearrange("(t p) j -> p t j", p=128)
        # quarter 0 alone (small, unblocks the first S matmuls), quarter 1,
        # then 2-3 as one wide transfer (2KB+ per-partition segments).
        nc.sync.dma_start(out=cx_sb[:, :, 0:512], in_=ctxTr[:, :, 0:512])
        for b in range(BT):
            nc.sync.dma_start(out=wv_sb[:, b, :], in_=wvr[:, b, :])
        nc.sync.dma_start(out=cx_sb[:, :, 512:1024], in_=ctxTr[:, :, 512:1024])
        wqr = wq.rearrange("(t p) c -> p t c", p=128)
        xTr = xT.rearrange("(t p) i -> p t i", p=128)
        for a in range(AT):
            nc.sync.dma_start(out=wq_sb[:, a, :], in_=wqr[:, a, :])
            nc.sync.dma_start(out=xT_sb[:, a, :], in_=xTr[:, a, :])
        nc.sync.dma_start(out=cx_sb[:, :, 1024:2048], in_=ctxTr[:, :, 1024:2048])
        wor = wo.rearrange("(t p) e -> p t e", p=128)
        for t in range(CT):
            nc.sync.dma_start(out=wo_sb[:, t, :], in_=wor[:, t, :])

        # ones columns of [v_h | 1]
        for jb in range(JB):
            nc.vector.tensor_copy(v4[:, jb, :, 64:65], onesF[:, 0:H].unsqueeze(-1))
        # zero the dead half of each zero-padded head tile
        for t in range(CT):
            nc.vector.memset(qT_sb[64:128, 2 * t, :], 0.0)
            nc.vector.memset(qT_sb[0:64, 2 * t + 1, :], 0.0)
            nc.vector.memset(kT_sb[64:128, 2 * t, :], 0.0)
            nc.vector.memset(kT_sb[0:64, 2 * t + 1, :], 0.0)

        # ------------------------------------------------------------------
        # Projection work for head-pair p, as a list of (matmuls, evac)
        # closures so they can be interleaved into the previous pair's
        # attention loop.  Each group allocates one ps_pr tile.
        # ------------------------------------------------------------------
        def proj_groups(p):
            groups = []

            # k^T tile p:  k^T[c, j] via lhsT=Wk, rhs=ctx^T, per j-quarter
            # (quarter granularity so the first S matmuls start as soon as
            # ctx quarter 0 lands).  Evacuate each head's 64 rows into its
            # zero-padded tile.
            def kproj(jq, pool=None, tag="pr"):
                def run():
                    ps = (pool or ps_pr).tile(
                        [128, 512], F32, tag=tag, name="kps"
                    )
                    sl = slice(jq * 512, (jq + 1) * 512)
                    for b in range(BT):
                        nc.tensor.matmul(
                            ps,
                            lhsT=wk_sb[:, b, p * 128:(p + 1) * 128],
                            rhs=cx_sb[:, b, sl],
                            start=(b == 0),
                            stop=(b == BT - 1),
                        )
                    nc.vector.tensor_copy(
                        kT_sb[0:64, 2 * p, sl], ps[0:64, :]
                    )
                    nc.vector.tensor_copy(
                        kT_sb[64:128, 2 * p + 1, sl], ps[64:128, :]
                    )
                return run

            # q^T tile p
            def qproj():
                def run():
                    ps = ps_pr.tile([128, NQ], F32, tag="pr")
                    for a in range(AT):
                        for c2 in range(2):
                            nc.tensor.matmul(
                                ps[:, c2 * 512:(c2 + 1) * 512],
                                lhsT=wq_sb[:, a, p * 128:(p + 1) * 128],
                                rhs=xT_sb[:, a, c2 * 512:(c2 + 1) * 512],
                                start=(a == 0),
                                stop=(a == AT - 1),
                            )
                    nc.vector.tensor_copy(qT_sb[0:64, 2 * p, :], ps[0:64, :])
                    nc.vector.tensor_copy(
                        qT_sb[64:128, 2 * p + 1, :], ps[64:128, :]
                    )
                return run

            if p == 0:
                # prelude groups, ordered to match DMA arrival; the PV-acc
                # banks are still free so the k^T groups pipeline.
                groups.append(kproj(0, pool=ps_acc, tag="acc0"))
                groups.append(kproj(1, pool=ps_acc, tag="acc1"))
                groups.append(qproj())
                groups.append(kproj(2))
                groups.append(kproj(3))
            else:
                for jq in range(4):
                    groups.append(kproj(jq))
                groups.append(qproj())
            return groups

        # v columns for head-pairs [p0, p1): per jb, [128 j, (p1-p0)*128 c]
        def vproj_groups(p0, p1):
            w = (p1 - p0) * 128

            def vproj(jb):
                def run():
                    ps = ps_pr.tile([128, w], F32, tag="pr", name="vps")
                    for b in range(BT):
                        nc.tensor.matmul(
                            ps,
                            lhsT=cx_sb[:, b, jb * 128:(jb + 1) * 128],
                            rhs=wv_sb[:, b, p0 * 128:p1 * 128],
                            start=(b == 0),
                            stop=(b == BT - 1),
                        )
                    nc.vector.tensor_copy(
                        v4[:, jb, 2 * p0:2 * p1, 0:64],
                        ps.rearrange("p (h d) -> p h d", d=DH),
                    )
                return run

            return [vproj(jb) for jb in range(JB)]

        # ------------------------------------------------------------------
        # Attention for head-pair p (heads 2p / 2p+1), interleaving the
        # projection groups of pair p+1.
        # ------------------------------------------------------------------
        def attention(p, interleave):
            il = iter(interleave)
            n_emitted = 0

            def tick(budget):
                nonlocal n_emitted
                for _ in range(budget):
                    g = next(il, None)
                    if g is None:
                        return
                    g()
                    n_emitted += 1

            osb = {}
            for hh in range(2):
                osb[hh] = opool.tile(
                    [65, NQ], F32, tag=f"osb{hh}", name=f"osb{hh}"
                )
            dcol = rpool.tile([2, NQ], F32, tag="dcol")
            r1 = rpool.tile([1, NQ], F32, tag="r1")
            rb = {}
            rb[0] = rpool.tile([64, NQ], F32, tag="rb0", name="rb0")
            rb[1] = rpool.tile([64, NQ], F32, tag="rb1", name="rb1")
            ot = otmp.tile([64, NQ], BF16, tag="ot")

            LAG = 2  # PV trails exp by LAG iters so the PE never waits on ACT

            for ch in range(2):  # i-halves
                acc = {}
                for hh in range(2):
                    acc[hh] = ps_acc.tile(
                        [65, 512], F32, tag=f"acc{hh}", name=f"acc{hh}"
                    )
                es_q = []

                def pv(jb, es):
                    for hh in range(2):
                        nc.tensor.matmul(
                            acc[hh][0:65, :],
                            lhsT=v4[:, jb, 2 * p + hh, :],
                            rhs=es[:, hh * 512:(hh + 1) * 512],
                            start=(jb == 0),
                            stop=(jb == JB - 1),
                        )

                for jb in range(JB):
                    tick(1)
                    sq = ps_s.tile([128, 1024], F32, tag="s")
                    for hh in range(2):
                        nc.tensor.matmul(
                            sq[:, hh * 512:(hh + 1) * 512],
                            lhsT=kT_sb[:, 2 * p + hh,
                                       jb * 128:(jb + 1) * 128],
                            rhs=qT_sb[:, 2 * p + hh,
                                      ch * 512:(ch + 1) * 512],
                            start=True,
                            stop=True,
                        )
                    es = expp.tile([128, 1024], BF16, tag="es")
                    nc.scalar.activation(
                        es, sq, mybir.ActivationFunctionType.Exp, scale=SCALE
                    )
                    es_q.append((jb, es))
                    if len(es_q) > LAG:
                        pv(*es_q.pop(0))
                for it in es_q:
                    pv(*it)
                # ---- evacuate + normalize this i-half (off the PE; the
                # ch=0 chain overlaps the ch=1 attention) ----
                sl = slice(ch * 512, (ch + 1) * 512)
                for hh in range(2):
                    nc.vector.tensor_copy(osb[hh][:, sl], acc[hh])
                for hh in range(2):
                    nc.sync.dma_start(
                        out=dcol[hh:hh + 1, sl], in_=osb[hh][64:65, sl]
                    )
                nc.vector.reciprocal_approx_fast(
                    out=dcol[:, sl], in_=dcol[:, sl]
                )
                nc.sync.dma_start(out=r1[0:1, sl], in_=dcol[1:2, sl])
                nc.gpsimd.partition_broadcast(rb[0][:, sl], dcol[0:1, sl])
                nc.gpsimd.partition_broadcast(rb[1][:, sl], r1[0:1, sl])
                nc.vector.tensor_mul(
                    oT_sb[0:64, p, sl], osb[0][0:64, sl], rb[0][:, sl]
                )
                nc.vector.tensor_mul(ot[:, sl], osb[1][0:64, sl], rb[1][:, sl])
                nc.sync.dma_start(out=oT_sb[64:128, p, sl], in_=ot[:, sl])
            # drain any remaining interleave groups
            tick(1 << 30)

        # ------------------------------------------------------------------
        # Emit: prelude projections for pair 0, then pair-major attention
        # with later pairs' projections interleaved (v for pair p+1 must
        # land during pair p; k^T/q^T for pair p+1 likewise).
        # ------------------------------------------------------------------
        vall = vproj_groups(0, CT)  # all heads at once, N=512 matmuls
        pre = proj_groups(0)
        pre[0]()          # k^T(t0) quarter 0
        for g in vall[0:3]:
            g()
        pre[1]()          # k^T(t0) quarter 1
        pre[2]()          # q^T(t0)
        inter = {
            0: pre[3:] + vall[3:] + proj_groups(1),
            1: proj_groups(2),
            2: proj_groups(3),
            3: [],
        }
        for p in range(CT):
            attention(p, inter[p])

        # --- output projection: F = O^T.T @ Wo;  bias added on DVE ---
        for ib in range(IB):
            fp = ps_s.tile([128, 1024], F32, tag="s", name="fp")
            for c2 in range(2):
                for t in range(CT):
                    nc.tensor.matmul(
                        fp[:, c2 * 512:(c2 + 1) * 512],
                        lhsT=oT_sb[:, t, ib * 128:(ib + 1) * 128],
                        rhs=wo_sb[:, t, c2 * 512:(c2 + 1) * 512],
                        start=(t == 0),
                        stop=(t == CT - 1),
                    )
            ost = outp.tile([128, DQ], F32)
            nc.vector.tensor_add(ost, fp, bo_bc)
            nc.sync.dma_start(out=out[ib * 128:(ib + 1) * 128, :], in_=ost)


def get_program():
    if "nc" not in _CACHE:
        _CACHE["nc"] = _build_program()
    return _CACHE["nc"]


def make_in_maps(x, context, Wq, Wk, Wv, Wo, bo):
    bf = ml_dtypes.bfloat16
    in_maps = []
    wq_b = np.asarray(Wq).astype(bf)
    wk_b = np.asarray(Wk).astype(bf)
    wv_b = np.asarray(Wv).astype(bf)
    wo_b = np.asarray(Wo).astype(bf)
    bo_b = np.asarray(bo).astype(bf)
    for c in range(8):
        b, half = c // 2, c % 2
        in_maps.append({
            "xT": np.ascontiguousarray(
                x[b, half * NQ:(half + 1) * NQ, :].T
            ).astype(bf),
            "ctxT": np.ascontiguousarray(context[b].T).astype(bf),
            "Wq": wq_b,
            "Wk": wk_b,
            "Wv": wv_b,
            "Wo": wo_b,
            "bo": bo_b,
        })
    return in_maps


def kernel(x, context, Wq, Wk, Wv, Wo, bo):
    nc = get_program()
    in_maps = make_in_maps(x, context, Wq, Wk, Wv, Wo, bo)
    res = run_bass_kernel_spmd(nc, in_maps, list(range(8)))
    out = np.empty((B, NQ_FULL, DQ), np.float32)
    for c in range(8):
        b, half = c // 2, c % 2
        out[b, half * NQ:(half + 1) * NQ, :] = res.results[c]["out"]
    return out


# revision 35
# speedup vs baseline: 1.5330x; 1.0226x over previous
"""Trainium2 Bass kernel for multi-head cross-attention.

Reference computation (fp32):
  q = x @ Wq; k = ctx @ Wk; v = ctx @ Wv              (per batch)
  sim = einsum('bihd,bjhd->bhij', q, k) * 1/sqrt(64)
  out = softmax(sim) @ v ; out = out @ Wo + bo

Shapes: x (4, 2048, 1024), context (4, 2048, 768), HEADS=8, DIM_HEAD=64.

Sharding: 8 cores = (batch b = core//2) x (query half = core%2). Each core
computes the full attention for its 1024 query rows across all 8 heads with
replicated weights; outputs concatenate - no cross-core reduction.

On-core dataflow (v2, pair-major, ACT-paced):
  - Heads are processed in PAIRS (2t, 2t+1) sharing feature tile t: the even
    head lives on SBUF partitions 0-63, the odd head on 64-127. The two QK^T
    matmuls of a pair have K=64 and are issued as row-tiled 64x128 PE tiles
    (tile_position (0,0) / (64,0)), so they run CONCURRENTLY - 2x throughput
    on the score phase.
  - Scores for (pair, i-half, jb, jb+1) land in one [128, 2048] PSUM tile
    (4 banks); ONE ACT exp (scale=1/8 folded) converts it to bf16 es - large
    ACT calls amortize the ~300-cycle ACTIVATE overhead (ACT is the pacing
    engine: 2048*1024*8 exps/core ~= 109us of ACT at 1 elem/lane/cycle).
  - PV accumulates per (pair, i-half): lhsT=[v_h|1] (65 cols: 64 dims +
    softmax-denominator ones column) into [65, 512] PSUM accs (1 bank each).
  - PSUM budget: S-quad 4 banks + 2 PV accs + 2 proj banks = 8 exactly. The
    spare 2 banks let projections for pair p+1 (Wq/Wk/Wv matmuls) interleave
    INTO pair p's attention loop, filling the PE while ACT paces the loop.
  - Normalize per pair off the PE: denominators staged via lane-shift DMAs
    into one [2, 1024] tile -> ONE batched DVE reciprocal (a [1,1024]
    single-lane reciprocal measures 6.5us on HW; batching pairs halves it),
    gpsimd partition_broadcast, DVE multiply. Odd head is lane-shifted into
    the stacked O^T layout via SBUF->SBUF DMA (DVE is lane-locked).
  - Final projection F = O^T.T @ Wo; bias is added on the DVE during PSUM
    evacuation from a gpsimd-broadcast [128, 1024] bias tile (no PE bias
    matmuls).
"""

import ml_dtypes
import numpy as np

import concourse.bass as bass
import concourse.tile as tile
from concourse import bacc, mybir
from concourse.bass_utils import run_bass_kernel_spmd

F32 = mybir.dt.float32
BF16 = mybir.dt.bfloat16

B = 4
NQ_FULL = 2048
NQ = 1024  # local query rows per core
NC = 2048
DQ = 1024
DC = 768
H = 8
DH = 64
INNER = H * DH  # 512
SCALE = DH ** -0.5

AT = DQ // 128   # 8  k-tiles of the q-projection contraction
BT = DC // 128   # 6  k-tiles of the k/v-projection contraction
CT = INNER // 128  # 4 feature tiles of q^T/k^T/o^T (= head pairs)
IB = NQ // 128   # 8  query-row blocks
JB = NC // 128   # 16 context-row blocks

_CACHE = {}


def _build_program():
    nc = bacc.Bacc(
        "TRN2",
        target_bir_lowering=False,
        debug=False,
        enable_asserts=False,
    )

    xT = nc.dram_tensor("xT", [DQ, NQ], BF16, kind="ExternalInput").ap()
    ctxT = nc.dram_tensor("ctxT", [DC, NC], BF16, kind="ExternalInput").ap()
    wq = nc.dram_tensor("Wq", [DQ, INNER], BF16, kind="ExternalInput").ap()
    wk = nc.dram_tensor("Wk", [DC, INNER], BF16, kind="ExternalInput").ap()
    wv = nc.dram_tensor("Wv", [DC, INNER], BF16, kind="ExternalInput").ap()
    wo = nc.dram_tensor("Wo", [INNER, DQ], BF16, kind="ExternalInput").ap()
    bo = nc.dram_tensor("bo", [DQ], BF16, kind="ExternalInput").ap()
    out = nc.dram_tensor("out", [NQ, DQ], F32, kind="ExternalOutput").ap()

    with tile.TileContext(nc) as tc:
        with nc.allow_low_precision(reason="bf16 matmul operands"):
            _emit(nc, tc, xT, ctxT, wq, wk, wv, wo, bo, out)

    nc.compile()
    return nc


def _emit(nc, tc, xT, ctxT, wq, wk, wv, wo, bo, out):
    from contextlib import ExitStack

    with ExitStack() as ctx:
        const = ctx.enter_context(tc.tile_pool(name="const", bufs=1))
        persist = ctx.enter_context(tc.tile_pool(name="persist", bufs=1))
        expp = ctx.enter_context(tc.tile_pool(name="expp", bufs=4))
        opool = ctx.enter_context(tc.tile_pool(name="opool", bufs=1))
        rpool = ctx.enter_context(tc.tile_pool(name="rpool", bufs=1))
        otmp = ctx.enter_context(tc.tile_pool(name="otmp", bufs=1))
        outp = ctx.enter_context(tc.tile_pool(name="outp", bufs=2))
        # PSUM: 8 banks total.  S tiles 2x2 + two PV accs 1+1 + proj 2 = 8.
        ps_s = ctx.enter_context(tc.tile_pool(name="ps_s", bufs=2, space="PSUM"))
        ps_acc = ctx.enter_context(tc.tile_pool(name="ps_acc", bufs=1, space="PSUM"))
        ps_pr = ctx.enter_context(tc.tile_pool(name="ps_pr", bufs=1, space="PSUM"))

        # --- constants ---
        bo_sb = const.tile([1, DQ], BF16)
        nc.sync.dma_start(out=bo_sb, in_=bo.unsqueeze(0))
        onesF = const.tile([128, 128], F32)
        nc.vector.memset(onesF, 1.0)
        bo_bc = const.tile([128, DQ], BF16)
        nc.gpsimd.partition_broadcast(bo_bc, bo_sb)

        # --- persistent SBUF tensors ---
        xT_sb = persist.tile([128, AT, NQ], BF16)      # 16 KB/part
        cx_sb = persist.tile([128, BT, NC], BF16)      # 24 KB/part
        wq_sb = persist.tile([128, AT, INNER], BF16)   # 8 KB
        wk_sb = persist.tile([128, BT, INNER], BF16)   # 6 KB
        wv_sb = persist.tile([128, BT, INNER], BF16)   # 6 KB
        wo_sb = persist.tile([128, CT, DQ], BF16)      # 8 KB
        # Per-head zero-padded q^T / k^T: head h occupies its native 64
        # partitions (even: 0-63, odd: 64-127); the other 64 rows are zeroed
        # once.  This lets the QK^T matmuls run as full K=128 128x128-mode
        # matmuls - no 64-row PE tiling modes, so the whole kernel stays in
        # one PE mode (no mode-switch drains) and LDWEIGHTS pipelines behind
        # the previous matmul via the background weight buffer.
        qT_sb = persist.tile([128, H, NQ], BF16)       # 16 KB
        kT_sb = persist.tile([128, H, NC], BF16)       # 32 KB
        v_sb = persist.tile([128, JB, H * 65], BF16)   # 16.3 KB
        oT_sb = persist.tile([128, CT, NQ], BF16)      # 8 KB

        v4 = v_sb.rearrange("p j (h e) -> p j h e", e=65)

        # --- input DMAs, in consumption order.  One wide dma_start per
        # tensor (or large slice): each dma_start costs ~650ns of serial
        # dispatch on the Sync engine, so fewer+bigger wins. ---
        wkr = wk.rearrange("(t p) c -> p t c", p=128)
        wvr = wv.rearrange("(t p) c -> p t c", p=128)
        nc.sync.dma_start(out=wk_sb, in_=wkr)
        ctxTr = ctxT.rearrange("(t p) j -> p t j", p=128)
        # quarter 0 alone (small, unblocks the first S matmuls), then the
        # rest as wide transfers (2KB+ per-partition segments).
        nc.sync.dma_start(out=cx_sb[:, :, 0:512], in_=ctxTr[:, :, 0:512])
        nc.sync.dma_start(out=wv_sb, in_=wvr)
        nc.sync.dma_start(out=cx_sb[:, :, 512:1024], in_=ctxTr[:, :, 512:1024])
        wqr = wq.rearrange("(t p) c -> p t c", p=128)
        xTr = xT.rearrange("(t p) i -> p t i", p=128)
        nc.sync.dma_start(out=wq_sb, in_=wqr)
        nc.sync.dma_start(out=xT_sb[:, 0:4, :], in_=xTr[:, 0:4, :])
        nc.sync.dma_start(out=xT_sb[:, 4:8, :], in_=xTr[:, 4:8, :])
        nc.sync.dma_start(out=cx_sb[:, :, 1024:2048], in_=ctxTr[:, :, 1024:2048])
        wor = wo.rearrange("(t p) e -> p t e", p=128)
        nc.sync.dma_start(out=wo_sb, in_=wor)

        # ones columns of [v_h | 1]
        for jb in range(JB):
            nc.vector.tensor_copy(v4[:, jb, :, 64:65], onesF[:, 0:H].unsqueeze(-1))
        # zero the dead half of each zero-padded head tile
        for t in range(CT):
            nc.vector.memset(qT_sb[64:128, 2 * t, :], 0.0)
            nc.vector.memset(qT_sb[0:64, 2 * t + 1, :], 0.0)
            nc.vector.memset(kT_sb[64:128, 2 * t, :], 0.0)
            nc.vector.memset(kT_sb[0:64, 2 * t + 1, :], 0.0)

        # ------------------------------------------------------------------
        # Projection work for head-pair p, as a list of (matmuls, evac)
        # closures so they can be interleaved into the previous pair's
        # attention loop.  Each group allocates one ps_pr tile.
        # ------------------------------------------------------------------
        def proj_groups(p):
            groups = []

            # k^T tile p:  k^T[c, j] via lhsT=Wk, rhs=ctx^T, per j-quarter
            # (quarter granularity so the first S matmuls start as soon as
            # ctx quarter 0 lands).  Evacuate each head's 64 rows into its
            # zero-padded tile.
            def kproj(jq, pool=None, tag="pr"):
                def run():
                    ps = (pool or ps_pr).tile(
                        [128, 512], F32, tag=tag, name="kps"
                    )
                    sl = slice(jq * 512, (jq + 1) * 512)
                    for b in range(BT):
                        nc.tensor.matmul(
                            ps,
                            lhsT=wk_sb[:, b, p * 128:(p + 1) * 128],
                            rhs=cx_sb[:, b, sl],
                            start=(b == 0),
                            stop=(b == BT - 1),
                        )
                    nc.vector.tensor_copy(
                        kT_sb[0:64, 2 * p, sl], ps[0:64, :]
                    )
                    nc.vector.tensor_copy(
                        kT_sb[64:128, 2 * p + 1, sl], ps[64:128, :]
                    )
                return run

            # q^T tile p
            def qproj():
                def run():
                    ps = ps_pr.tile([128, NQ], F32, tag="pr")
                    for a in range(AT):
                        for c2 in range(2):
                            nc.tensor.matmul(
                                ps[:, c2 * 512:(c2 + 1) * 512],
                                lhsT=wq_sb[:, a, p * 128:(p + 1) * 128],
                                rhs=xT_sb[:, a, c2 * 512:(c2 + 1) * 512],
                                start=(a == 0),
                                stop=(a == AT - 1),
                            )
                    nc.vector.tensor_copy(qT_sb[0:64, 2 * p, :], ps[0:64, :])
                    nc.vector.tensor_copy(
                        qT_sb[64:128, 2 * p + 1, :], ps[64:128, :]
                    )
                return run

            if p == 0:
                # prelude groups, ordered to match DMA arrival; the PV-acc
                # banks are still free so the k^T groups pipeline.
                groups.append(kproj(0, pool=ps_acc, tag="acc0"))
                groups.append(kproj(1, pool=ps_acc, tag="acc1"))
                groups.append(qproj())
                groups.append(kproj(2))
                groups.append(kproj(3))
            else:
                for jq in range(4):
                    groups.append(kproj(jq))
                groups.append(qproj())
            return groups

        # v columns for head-pairs [p0, p1): per jb, [128 j, (p1-p0)*128 c]
        def vproj_groups(p0, p1):
            w = (p1 - p0) * 128

            def vproj(jb):
                def run():
                    ps = ps_pr.tile([128, w], F32, tag="pr", name="vps")
                    for b in range(BT):
                        nc.tensor.matmul(
                            ps,
                            lhsT=cx_sb[:, b, jb * 128:(jb + 1) * 128],
                            rhs=wv_sb[:, b, p0 * 128:p1 * 128],
                            start=(b == 0),
                            stop=(b == BT - 1),
                        )
                    nc.vector.tensor_copy(
                        v4[:, jb, 2 * p0:2 * p1, 0:64],
                        ps.rearrange("p (h d) -> p h d", d=DH),
                    )
                return run

            return [vproj(jb) for jb in range(JB)]

        # ------------------------------------------------------------------
        # Attention for head-pair p (heads 2p / 2p+1), interleaving the
        # projection groups of pair p+1.
        # ------------------------------------------------------------------
        def attention(p, interleave):
            il = iter(interleave)
            n_emitted = 0

            def tick(budget):
                nonlocal n_emitted
                for _ in range(budget):
                    g = next(il, None)
                    if g is None:
                        return
                    g()
                    n_emitted += 1

            osb = {}
            for hh in range(2):
                osb[hh] = opool.tile(
                    [65, NQ], F32, tag=f"osb{hh}", name=f"osb{hh}"
                )
            dcol = rpool.tile([2, NQ], F32, tag="dcol")
            r1 = rpool.tile([1, NQ], F32, tag="r1")
            rb = {}
            rb[0] = rpool.tile([64, NQ], F32, tag="rb0", name="rb0")
            rb[1] = rpool.tile([64, NQ], F32, tag="rb1", name="rb1")
            ot = otmp.tile([64, NQ], BF16, tag="ot")

            LAG = 2  # PV trails exp by LAG iters so the PE never waits on ACT

            for ch in range(2):  # i-halves
                acc = {}
                for hh in range(2):
                    acc[hh] = ps_acc.tile(
                        [65, 512], F32, tag=f"acc{hh}", name=f"acc{hh}"
                    )
                es_q = []

                def pv(jb, es):
                    for hh in range(2):
                        nc.tensor.matmul(
                            acc[hh][0:65, :],
                            lhsT=v4[:, jb, 2 * p + hh, :],
                            rhs=es[:, hh * 512:(hh + 1) * 512],
                            start=(jb == 0),
                            stop=(jb == JB - 1),
                        )

                for jb in range(JB):
                    tick(1)
                    sq = ps_s.tile([128, 1024], F32, tag="s")
                    for hh in range(2):
                        nc.tensor.matmul(
                            sq[:, hh * 512:(hh + 1) * 512],
                            lhsT=kT_sb[:, 2 * p + hh,
                                       jb * 128:(jb + 1) * 128],
                            rhs=qT_sb[:, 2 * p + hh,
                                      ch * 512:(ch + 1) * 512],
                            start=True,
                            stop=True,
                        )
                    es = expp.tile([128, 1024], BF16, tag="es")
                    nc.scalar.activation(
                        es, sq, mybir.ActivationFunctionType.Exp, scale=SCALE
                    )
                    es_q.append((jb, es))
                    if len(es_q) > LAG:
                        pv(*es_q.pop(0))
                for it in es_q:
                    pv(*it)
                # ---- evacuate + normalize this i-half (off the PE; the
                # ch=0 chain overlaps the ch=1 attention) ----
                sl = slice(ch * 512, (ch + 1) * 512)
                for hh in range(2):
                    nc.vector.tensor_copy(osb[hh][:, sl], acc[hh])
                for hh in range(2):
                    nc.sync.dma_start(
                        out=dcol[hh:hh + 1, sl], in_=osb[hh][64:65, sl]
                    )
                nc.vector.reciprocal_approx_fast(
                    out=dcol[:, sl], in_=dcol[:, sl]
                )
                nc.sync.dma_start(out=r1[0:1, sl], in_=dcol[1:2, sl])
                nc.gpsimd.partition_broadcast(rb[0][:, sl], dcol[0:1, sl])
                nc.gpsimd.partition_broadcast(rb[1][:, sl], r1[0:1, sl])
                nc.vector.tensor_mul(
                    oT_sb[0:64, p, sl], osb[0][0:64, sl], rb[0][:, sl]
                )
                nc.vector.tensor_mul(ot[:, sl], osb[1][0:64, sl], rb[1][:, sl])
                nc.sync.dma_start(out=oT_sb[64:128, p, sl], in_=ot[:, sl])
            # drain any remaining interleave groups
            tick(1 << 30)

        # ------------------------------------------------------------------
        # Emit: prelude projections for pair 0, then pair-major attention
        # with later pairs' projections interleaved (v for pair p+1 must
        # land during pair p; k^T/q^T for pair p+1 likewise).
        # ------------------------------------------------------------------
        vall = vproj_groups(0, CT)  # all heads at once, N=512 matmuls
        pre = proj_groups(0)
        pre[0]()          # k^T(t0) quarter 0
        for g in vall[0:3]:
            g()
        pre[1]()          # k^T(t0) quarter 1
        pre[2]()          # q^T(t0)
        inter = {
            0: pre[3:] + vall[3:] + proj_groups(1),
            1: proj_groups(2),
            2: proj_groups(3),
            3: [],
        }
        for p in range(CT):
            attention(p, inter[p])

        # --- output projection: F = O^T.T @ Wo;  bias added on DVE ---
        for ib in range(IB):
            fp = ps_s.tile([128, 1024], F32, tag="s", name="fp")
            for c2 in range(2):
                for t in range(CT):
                    nc.tensor.matmul(
                        fp[:, c2 * 512:(c2 + 1) * 512],
                        lhsT=oT_sb[:, t, ib * 128:(ib + 1) * 128],
                        rhs=wo_sb[:, t, c2 * 512:(c2 + 1) * 512],
                        start=(t == 0),
                        stop=(t == CT - 1),
                    )
            ost = outp.tile([128, DQ], F32)
            nc.vector.tensor_add(ost, fp, bo_bc)
            nc.sync.dma_start(out=out[ib * 128:(ib + 1) * 128, :], in_=ost)


def get_program():
    if "nc" not in _CACHE:
        _CACHE["nc"] = _build_program()
    return _CACHE["nc"]


def make_in_maps(x, context, Wq, Wk, Wv, Wo, bo):
    bf = ml_dtypes.bfloat16
    in_maps = []
    wq_b = np.asarray(Wq).astype(bf)
    wk_b = np.asarray(Wk).astype(bf)
    wv_b = np.asarray(Wv).astype(bf)
    wo_b = np.asarray(Wo).astype(bf)
    bo_b = np.asarray(bo).astype(bf)
    for c in range(8):
        b, half = c // 2, c % 2
        in_maps.append({
            "xT": np.ascontiguousarray(
                x[b, half * NQ:(half + 1) * NQ, :].T
            ).astype(bf),
            "ctxT": np.ascontiguousarray(context[b].T).astype(bf),
            "Wq": wq_b,
            "Wk": wk_b,
            "Wv": wv_b,
            "Wo": wo_b,
            "bo": bo_b,
        })
    return in_maps


def kernel(x, context, Wq, Wk, Wv, Wo, bo):
    nc = get_program()
    in_maps = make_in_maps(x, context, Wq, Wk, Wv, Wo, bo)
    res = run_bass_kernel_spmd(nc, in_maps, list(range(8)))
    out = np.empty((B, NQ_FULL, DQ), np.float32)
    for c in range(8):
        b, half = c // 2, c % 2
        out[b, half * NQ:(half + 1) * NQ, :] = res.results[c]["out"]
    return out
